# revision 6
# baseline (speedup 1.0000x reference)
"""Trainium2 Bass kernel for nn_CalibrationModelObsGridGeometry.

Single fused SPMD launch on 8 cores (3 swaths/core), minimal host<->device
traffic (the axon tunnel is ~10 MB/s, so bytes moved dominate wall time):

  host:   gather + replicate-pad fy/fs, cast fp16               (~0.75 MB/core)
  device: window via DMA -> gaussian-pyramid Toeplitz matmuls (fp16) ->
          per-core BN partial sums -> 8-core AllReduce (96 B) ->
          fold stats into a per-partition scale/bias activation ->
          3x3 conv stack as accumulating matmuls (fp16, block-diag over
          4 row-quarters) -> fp16 output                        (~0.36 MB/core)
  host:   + fs_sel + scatter-add, mask.

Toeplitz bands, block-diagonal conv weights and biases are assembled
on-device from tiny uploads instead of shipping the expanded forms.
"""

import numpy as np

# ---------------------------------------------------------------- constants
B, P, H, W = 4, 8, 1200, 52
M_SEL, HI = 24, 1100
SIZE = 75
HALF = SIZE // 2  # 37
NS = (0.31446309894037083, 0.3886609494201447)
BN_EPS = 1e-5
HID = 32
NCORES = 8
SW = 3                      # swaths per core
NWIN = 21                   # toeplitz windows per swath (54 out rows each)
WJ = 54                     # out rows per window
NPAD = WJ * (NWIN - 1) + 128  # 1208 padded input rows
NQ = 4                      # h-quarters (partition groups)
QROWS = HI // NQ            # 275
NT = 5                      # processing tiles per swath
R = QROWS // NT             # 55 out rows per tile per quarter
W2 = 54                     # padded width
CAL_ROWS = R + 6            # 61 stored cal rows per tile
H1_ROWS = R + 4             # 59
H2_ROWS = R + 2             # 57
CAL_F = CAL_ROWS * W2       # 3294
H1_F = H1_ROWS * W2         # 3186
H2_F = H2_ROWS * W2         # 3078
O_F = R * W2                # 2970
CAL_SZ = CAL_F + 2          # +1 lead, +1 tail guard
H1_SZ = H1_F + 2
H2_SZ = H2_F + 2
CHUNK = 486                 # <=512 fp32 psum-bank limit
NST = SW * NT               # 15 processing tiles per core
NF = SW * W                 # 156
INVN = 1.0 / float(M_SEL * HI * W)
SCRROWS = 3 + WJ * NWIN + 3  # 1140 cal scratch rows (3 lead, tail garbage)

EMULATE = False             # numpy-emulate the device kernel (debug)


def _bands_from_kernel(kern):
    """12 cal channels as 75-tap bands: D0..D9, A(=G9 on fy), B(=G9 on fs)."""
    g = np.asarray(kern, np.float32).reshape(10, SIZE)
    bands = np.zeros((12, SIZE), np.float32)
    bands[0] = -g[0]
    bands[0, HALF] += 1.0
    for i in range(1, 10):
        bands[i] = g[i - 1] - g[i]
    bands[10] = g[9]
    bands[11] = g[9]
    return bands


def _chunks(total):
    out = []
    off = 0
    while off < total:
        sz = min(CHUNK, total - off)
        out.append((off, sz))
        off += sz
    return out


# ---------------------------------------------------------------- device build
_CACHE = {}


def _apply_tile_patch():
    import concourse.tile as tile
    from concourse import mybir
    from concourse.vector_clock import ScopedClock

    def _patched(self, tick_clock, wait_clock):
        nc = self.nc
        drain_inst = nc.sync.drain()
        wait_clock.add_sem_waits(
            drain_inst.ins, ScopedClock({None: tick_clock.global_clock})
        )
        si = drain_inst.ins.sync_info
        if si is not None and si.on_wait and len(si.on_wait) > 1:
            extra = list(si.on_wait[1:])
            del si.on_wait[1:]
            for w in extra:
                d2 = nc.sync.drain()
                si2 = d2.ins.sync_info
                if si2 is None:
                    d2.ins.sync_info = mybir.SyncInfo(on_wait=[w], on_update=[])
                else:
                    si2.on_wait.append(w)
        nc.all_engine_barrier()
        popped = nc._tile_sem_poison_stack.pop()
        assert popped is self._sem_poison
        nc.clear_and_free_semaphores(list(self.sems.allocated().values()))
        nc.all_engine_barrier()

    tile.TileContext._drain_and_barrier = _patched


_WSPLIT_N = [0]


def _split_waits(nc):
    """This walrus build accepts only one sync-wait per instruction: hoist
    extra waits onto same-engine NoOps placed just before the instruction."""
    from concourse import mybir
    for f in nc.m.functions:
        for bb in f.blocks:
            new_list = []
            for ins in bb.instructions:
                si = getattr(ins, "sync_info", None)
                if si is not None and si.on_wait and len(si.on_wait) > 1:
                    extra = list(si.on_wait[:-1])
                    del si.on_wait[:-1]
                    for w in extra:
                        _WSPLIT_N[0] += 1
                        nop = mybir.InstDrain(
                            name=f"WSPLIT-{_WSPLIT_N[0]}",
                            engine=ins.engine,
                            sync_info=mybir.SyncInfo(on_wait=[w], on_update=[]),
                            bass_is_fusable=False,
                        )
                        new_list.append(nop)
                new_list.append(ins)
            bb.instructions[:] = new_list


def _build():
    import concourse.bass as bass
    import concourse.tile as tile
    from concourse import mybir

    f32 = mybir.dt.float32
    f16 = mybir.dt.float16
    nc = bass.Bass("TRN2", num_devices=NCORES)
    fyp = nc.dram_tensor("fyp", [NPAD, NF], f16, kind="ExternalInput")
    fsp = nc.dram_tensor("fsp", [NPAD, NF], f16, kind="ExternalInput")
    bands = nc.dram_tensor("bands", [SIZE, 12, 1], f16, kind="ExternalInput")
    w1t = nc.dram_tensor("w1t", [9, 12, HID], f16, kind="ExternalInput")
    w2t = nc.dram_tensor("w2t", [9, HID, HID], f16, kind="ExternalInput")
    w3t = nc.dram_tensor("w3t", [9, HID, 1], f16, kind="ExternalInput")
    b1d = nc.dram_tensor("b1d", [HID, 1], f32, kind="ExternalInput")
    b2d = nc.dram_tensor("b2d", [HID, 1], f32, kind="ExternalInput")
    b3d = nc.dram_tensor("b3d", [1, 1], f32, kind="ExternalInput")
    o = nc.dram_tensor("o", [NST, 4, O_F], f16, kind="ExternalOutput")

    Relu = mybir.ActivationFunctionType.Relu
    Ident = mybir.ActivationFunctionType.Identity
    Square = mybir.ActivationFunctionType.Square
    Sqrt = mybir.ActivationFunctionType.Sqrt
    ADD = mybir.AluOpType.add
    MULT = mybir.AluOpType.mult
    SUB = mybir.AluOpType.subtract
    AXX = mybir.AxisListType.X

    with tile.TileContext(nc) as tc:
        with (
            tc.tile_pool(name="singles", bufs=1) as singles,
            tc.tile_pool(name="dram", bufs=1, space="DRAM") as dram,
        ):
            # ---------------- setup: windows, toeplitz, weights
            fyw_s = singles.tile([128, NWIN, NF], f16)
            fsw_s = singles.tile([128, NWIN, NF], f16)
            for w in range(NWIN):
                nc.sync.dma_start(out=fyw_s[:, w, :], in_=fyp[WJ * w:WJ * w + 128, :])
                nc.sync.dma_start(out=fsw_s[:, w, :], in_=fsp[WJ * w:WJ * w + 128, :])
            toep_s = singles.tile([128, 12, WJ], f16)
            nc.vector.memset(toep_s[:], 0.0)
            for j in range(WJ):
                nc.sync.dma_start(out=toep_s[j:j + SIZE, :, j:j + 1], in_=bands[:])
            l1s = singles.tile([48, 9, 128], f16)
            l2s = singles.tile([128, 9, 128], f16)
            l3s = singles.tile([128, 9, 4], f16)
            nc.vector.memset(l1s[:], 0.0)
            nc.vector.memset(l2s[:], 0.0)
            nc.vector.memset(l3s[:], 0.0)
            for t9 in range(9):
                for q in range(NQ):
                    nc.sync.dma_start(
                        out=l1s[12 * q:12 * q + 12, t9, 32 * q:32 * q + 32],
                        in_=w1t[t9])
                    nc.sync.dma_start(
                        out=l2s[32 * q:32 * q + 32, t9, 32 * q:32 * q + 32],
                        in_=w2t[t9])
                    nc.sync.dma_start(
                        out=l3s[32 * q:32 * q + 32, t9, q:q + 1],
                        in_=w3t[t9])
            b1s = singles.tile([128, 1], f32)
            b2s = singles.tile([128, 1], f32)
            b3s = singles.tile([4, 1], f32)
            for q in range(NQ):
                nc.sync.dma_start(out=b1s[32 * q:32 * q + 32, :], in_=b1d[:])
                nc.sync.dma_start(out=b2s[32 * q:32 * q + 32, :], in_=b2d[:])
                nc.sync.dma_start(out=b3s[q:q + 1, :], in_=b3d[:])
            ones54 = singles.tile([WJ, 1], f32)
            nc.vector.memset(ones54[:], 1.0)
            acc = singles.tile([WJ, 24], f32)
            nc.vector.memset(acc[:], 0.0)
            zrow = singles.tile([12, 3 * W2], f16)
            nc.vector.memset(zrow[:], 0.0)

            cal_d = dram.tile([12, SCRROWS, SW, W2], f16)
            stat_in = dram.tile([1, 24], f32)
            stat_out = dram.tile([1, 24], f32)
            mr_rd = dram.tile([NQ, 12, 1], f32)
            mr_nd = dram.tile([NQ, 12, 1], f32)

            # ---------------- phase A: gaussian pyramid + BN partial stats
            with (
                tc.tile_pool(name="stage", bufs=2) as stage,
                tc.tile_pool(name="psumA", bufs=4, space="PSUM") as psumA,
            ):
                for w in range(NWIN):
                    st = stage.tile([WJ, 12, NF], f16, tag="st")
                    for ch in range(12):
                        src = fsw_s if ch == 11 else fyw_s
                        ps = psumA.tile([WJ, NF], f32, tag="psA")
                        nc.tensor.matmul(
                            ps[:], lhsT=toep_s[:, ch, :], rhs=src[:, w, :],
                            start=True, stop=True)
                        nc.scalar.copy(st[:, ch, :], ps[:])
                    for s in range(SW):
                        nc.sync.dma_start(
                            out=cal_d[:, 3 + WJ * w:3 + WJ * w + WJ, s, 1:1 + W]
                            .rearrange("ch r c -> r ch c"),
                            in_=st[:, :, W * s:W * s + W])
                    vr = WJ if w < NWIN - 1 else HI - WJ * (NWIN - 1)  # 54 / 20
                    red1 = stage.tile([WJ, 12], f32, tag="red1")
                    nc.vector.tensor_reduce(
                        out=red1[0:vr], in_=st[0:vr], axis=AXX, op=ADD)
                    sq = stage.tile([WJ, 12, NF], f16, tag="sq")
                    nc.scalar.activation(out=sq[0:vr], in_=st[0:vr], func=Square)
                    red2 = stage.tile([WJ, 12], f32, tag="red2")
                    nc.vector.tensor_reduce(
                        out=red2[0:vr], in_=sq[0:vr], axis=AXX, op=ADD)
                    nc.vector.tensor_tensor(
                        out=acc[0:vr, 0:12], in0=acc[0:vr, 0:12],
                        in1=red1[0:vr], op=ADD)
                    nc.vector.tensor_tensor(
                        out=acc[0:vr, 12:24], in0=acc[0:vr, 12:24],
                        in1=red2[0:vr], op=ADD)

                # ---------------- BN stats: partition-sum, allreduce, fold
                ps1 = psumA.tile([1, 24], f32, tag="ps1")
                nc.tensor.matmul(ps1[:], lhsT=ones54[:], rhs=acc[:],
                                 start=True, stop=True)
                stat_sb = singles.tile([1, 24], f32)
                nc.scalar.copy(stat_sb[:], ps1[:])
            nc.gpsimd.dma_start(stat_in[:], stat_sb[:])
            nc.gpsimd.collective_compute(
                "AllReduce", ADD, replica_groups=[list(range(NCORES))],
                ins=[stat_in.opt()], outs=[stat_out.opt()])
            gl = singles.tile([1, 24], f32)
            nc.gpsimd.dma_start(gl[:], stat_out[:])
            mt = singles.tile([1, 12], f32)
            nc.vector.tensor_scalar_mul(mt[:], gl[:, 0:12], INVN)
            var = singles.tile([1, 12], f32)
            nc.vector.tensor_tensor(out=var[:], in0=mt[:], in1=mt[:], op=MULT)
            e2 = singles.tile([1, 12], f32)
            nc.vector.tensor_scalar_mul(e2[:], gl[:, 12:24], INVN)
            nc.vector.tensor_tensor(out=var[:], in0=e2[:], in1=var[:], op=SUB)
            eps_t = singles.tile([1, 1], f32)
            nc.vector.memset(eps_t[:], BN_EPS)
            sd = singles.tile([1, 12], f32)
            nc.scalar.activation(out=sd[:], in_=var[:], func=Sqrt,
                                 bias=eps_t[:, 0:1])
            rr_t = singles.tile([1, 12], f32)
            nc.vector.reciprocal(rr_t[:], sd[:])
            nmr = singles.tile([1, 12], f32)
            nc.vector.tensor_tensor(out=nmr[:], in0=mt[:], in1=rr_t[:], op=MULT)
            nc.vector.tensor_scalar_mul(nmr[:], nmr[:], -1.0)
            for q in range(NQ):
                nc.sync.dma_start(out=mr_rd[q:q + 1, :, 0], in_=rr_t[0:1, :])
                nc.sync.dma_start(out=mr_nd[q:q + 1, :, 0], in_=nmr[0:1, :])
            r48 = singles.tile([48, 1], f32)
            nc.sync.dma_start(
                out=r48[:], in_=mr_rd[:].rearrange("q c one -> (q c) one"))
            nmr48 = singles.tile([48, 1], f32)
            nc.sync.dma_start(
                out=nmr48[:], in_=mr_nd[:].rearrange("q c one -> (q c) one"))

            # ---------------- phase B: normalize + 3x3 conv stack
            with (
                tc.tile_pool(name="io", bufs=2) as io,
                tc.tile_pool(name="acts", bufs=2) as acts,
                tc.tile_pool(name="psum", bufs=6, space="PSUM") as psum,
                tc.tile_pool(name="psum3", bufs=2, space="PSUM") as psum3,
            ):
                for st_i in range(NST):
                    s_i, t_i = divmod(st_i, NT)
                    calr = io.tile([48, CAL_SZ], f16, tag="calr")
                    for q in range(NQ):
                        r0 = R * t_i + QROWS * q
                        nc.sync.dma_start(
                            out=calr[12 * q:12 * q + 12, 1:1 + CAL_F]
                            .rearrange("p (r c) -> p r c", c=W2),
                            in_=cal_d[:, r0:r0 + CAL_ROWS, s_i, :])
                    caln = io.tile([48, CAL_SZ], f16, tag="caln")
                    nc.scalar.activation(
                        out=caln[:], in_=calr[:], func=Ident,
                        scale=r48[:, 0:1], bias=nmr48[:, 0:1])
                    cv = caln[:, 1:1 + CAL_F].rearrange("p (r c) -> p r c", c=W2)
                    nc.vector.memset(cv[:, :, 0:1], 0.0)
                    nc.vector.memset(cv[:, :, W2 - 1:W2], 0.0)
                    nc.vector.memset(caln[:, 0:1], 0.0)
                    nc.vector.memset(caln[:, 1 + CAL_F:], 0.0)
                    if t_i == 0:      # swath top: zero pad rows of quarter 0
                        nc.vector.memset(caln[0:12, 1:1 + 3 * W2], 0.0)
                    if t_i == NT - 1:  # swath bottom: zero pad rows of quarter 3
                        # (DMA: compute-engine APs need 32-aligned partition start)
                        nc.sync.dma_start(
                            out=caln[36:48, 1 + (CAL_ROWS - 3) * W2:1 + CAL_F],
                            in_=zrow[:])

                    h1 = acts.tile([128, H1_SZ], f16, tag="h1")
                    h2 = acts.tile([128, H2_SZ], f16, tag="h2")
                    ot = io.tile([4, O_F], f16, tag="ot")

                    # ---- conv1: caln[48] -> h1[128], ReLU(. + b1)
                    for off, sz in _chunks(H1_F):
                        ps = psum.tile([128, CHUNK], f32, tag="ps")
                        for t9 in range(9):
                            dy, dx = t9 // 3 - 1, t9 % 3 - 1
                            base = off + W2 * (1 + dy) + dx + 1
                            nc.tensor.matmul(
                                ps[:, :sz], lhsT=l1s[:, t9, :],
                                rhs=caln[:, base:base + sz],
                                start=(t9 == 0), stop=(t9 == 8),
                            )
                        nc.scalar.activation(
                            out=h1[:, 1 + off:1 + off + sz], in_=ps[:, :sz],
                            func=Relu, bias=b1s[:, 0:1], scale=1.0,
                        )
                    h1v = h1[:, 1:1 + H1_F].rearrange("p (r c) -> p r c", c=W2)
                    nc.vector.memset(h1v[:, :, 0:1], 0.0)
                    nc.vector.memset(h1v[:, :, W2 - 1:W2], 0.0)
                    if t_i == 0:
                        nc.vector.memset(h1[0:32, 1:1 + 2 * W2], 0.0)
                    if t_i == NT - 1:
                        nc.vector.memset(
                            h1[96:128, 1 + (H1_ROWS - 2) * W2:1 + H1_F], 0.0)

                    # ---- conv2: h1[128] -> h2[128], ReLU(. + b2)
                    for off, sz in _chunks(H2_F):
                        ps = psum.tile([128, CHUNK], f32, tag="ps")
                        for t9 in range(9):
                            dy, dx = t9 // 3 - 1, t9 % 3 - 1
                            base = off + W2 * (1 + dy) + dx + 1
                            nc.tensor.matmul(
                                ps[:, :sz], lhsT=l2s[:, t9, :],
                                rhs=h1[:, base:base + sz],
                                start=(t9 == 0), stop=(t9 == 8),
                            )
                        nc.scalar.activation(
                            out=h2[:, 1 + off:1 + off + sz], in_=ps[:, :sz],
                            func=Relu, bias=b2s[:, 0:1], scale=1.0,
                        )
                    h2v = h2[:, 1:1 + H2_F].rearrange("p (r c) -> p r c", c=W2)
                    nc.vector.memset(h2v[:, :, 0:1], 0.0)
                    nc.vector.memset(h2v[:, :, W2 - 1:W2], 0.0)
                    if t_i == 0:
                        nc.vector.memset(h2[0:32, 1:1 + W2], 0.0)
                    if t_i == NT - 1:
                        nc.vector.memset(
                            h2[96:128, 1 + (H2_ROWS - 1) * W2:1 + H2_F], 0.0)

                    # ---- conv3: h2[128] -> o[4], Identity(. + b3')
                    for off, sz in _chunks(O_F):
                        ps = psum3.tile([4, CHUNK], f32, tag="ps3")
                        for t9 in range(9):
                            dy, dx = t9 // 3 - 1, t9 % 3 - 1
                            base = off + W2 * (1 + dy) + dx + 1
                            nc.tensor.matmul(
                                ps[:, :sz], lhsT=l3s[:, t9, :],
                                rhs=h2[:, base:base + sz],
                                start=(t9 == 0), stop=(t9 == 8),
                            )
                        nc.scalar.activation(
                            out=ot[:, off:off + sz], in_=ps[:, :sz],
                            func=Ident, bias=b3s[:, 0:1], scale=1.0,
                        )
                    nc.sync.dma_start(out=o[st_i], in_=ot[:])
    _split_waits(nc)
    return nc


# ---------------------------------------------------------------- emulation
def _emulate(in_maps):
    """Numpy mirror of the fused device kernel (fp16 casts at tile edges)."""
    f16 = np.float16
    stats = np.zeros((1, 24), np.float32)
    cores = []
    for m in in_maps:
        fyp = m["fyp"].astype(np.float32)
        fsp = m["fsp"].astype(np.float32)
        bands = m["bands"].astype(np.float32).reshape(SIZE, 12)
        toep = np.zeros((128, 12, WJ), np.float32)
        for j in range(WJ):
            toep[j:j + SIZE, :, j] = bands
        cal_d = np.zeros((12, SCRROWS, SW, W2), np.float32)
        acc = np.zeros((WJ, 24), np.float32)
        for w in range(NWIN):
            st = np.zeros((WJ, 12, NF), np.float32)
            for ch in range(12):
                src = fsp if ch == 11 else fyp
                st[:, ch, :] = toep[:, ch, :].T @ src[WJ * w:WJ * w + 128, :]
            st = st.astype(f16).astype(np.float32)
            cal_d[:, 3 + WJ * w:3 + WJ * w + WJ, :, 1:1 + W] = (
                st.reshape(WJ, 12, SW, W).transpose(1, 0, 2, 3))
            vr = WJ if w < NWIN - 1 else HI - WJ * (NWIN - 1)
            acc[0:vr, 0:12] += st[0:vr].sum(2)
            sq = (st[0:vr] ** 2).astype(f16).astype(np.float32)
            acc[0:vr, 12:24] += sq.sum(2)
        stats += acc.sum(0, keepdims=True)
        cores.append(cal_d)
    mt = stats[:, 0:12] * INVN
    var = stats[:, 12:24] * INVN - mt * mt
    rr = 1.0 / np.sqrt(var + BN_EPS)
    nmr = -(mt * rr)
    r48 = np.tile(rr[0], NQ)[:, None]
    nmr48 = np.tile(nmr[0], NQ)[:, None]

    outs = []
    for m, cal_d in zip(in_maps, cores):
        l1 = np.zeros((9, 48, 128), np.float32)
        l2 = np.zeros((9, 128, 128), np.float32)
        l3 = np.zeros((9, 128, 4), np.float32)
        w1t = m["w1t"].astype(np.float32)
        w2t = m["w2t"].astype(np.float32)
        w3t = m["w3t"].astype(np.float32)
        for t9 in range(9):
            for q in range(NQ):
                l1[t9, 12 * q:12 * q + 12, 32 * q:32 * q + 32] = w1t[t9]
                l2[t9, 32 * q:32 * q + 32, 32 * q:32 * q + 32] = w2t[t9]
                l3[t9, 32 * q:32 * q + 32, q] = w3t[t9][:, 0]
        b1s = np.tile(m["b1d"][:, 0], NQ)[:, None]
        b2s = np.tile(m["b2d"][:, 0], NQ)[:, None]
        b3s = np.full((4, 1), m["b3d"][0, 0], np.float32)
        o = np.zeros((NST, 4, O_F), np.float32)
        for st_i in range(NST):
            s_i, t_i = divmod(st_i, NT)
            calr = np.zeros((48, CAL_SZ), np.float32)
            for q in range(NQ):
                r0 = R * t_i + QROWS * q
                calr[12 * q:12 * q + 12, 1:1 + CAL_F] = (
                    cal_d[:, r0:r0 + CAL_ROWS, s_i, :].reshape(12, CAL_F))
            caln = (calr * r48 + nmr48).astype(f16).astype(np.float32)
            cv = caln[:, 1:1 + CAL_F].reshape(48, CAL_ROWS, W2)
            cv[:, :, 0] = 0.0
            cv[:, :, W2 - 1] = 0.0
            caln[:, 0] = 0.0
            caln[:, 1 + CAL_F:] = 0.0
            if t_i == 0:
                caln[0:12, 1:1 + 3 * W2] = 0.0
            if t_i == NT - 1:
                caln[36:48, 1 + (CAL_ROWS - 3) * W2:1 + CAL_F] = 0.0
            h1 = np.zeros((128, H1_SZ), np.float32)
            acc9 = np.zeros((128, H1_F), np.float32)
            for t9 in range(9):
                dy, dx = t9 // 3 - 1, t9 % 3 - 1
                base = W2 * (1 + dy) + dx + 1
                acc9 += l1[t9].T @ caln[:, base:base + H1_F]
            h1[:, 1:1 + H1_F] = np.maximum(acc9 + b1s, 0.0)
            h1 = h1.astype(f16).astype(np.float32)
            h1v = h1[:, 1:1 + H1_F].reshape(128, H1_ROWS, W2)
            h1v[:, :, 0] = 0.0
            h1v[:, :, W2 - 1] = 0.0
            if t_i == 0:
                h1[0:32, 1:1 + 2 * W2] = 0.0
            if t_i == NT - 1:
                h1[96:128, 1 + (H1_ROWS - 2) * W2:1 + H1_F] = 0.0
            h2 = np.zeros((128, H2_SZ), np.float32)
            acc9 = np.zeros((128, H2_F), np.float32)
            for t9 in range(9):
                dy, dx = t9 // 3 - 1, t9 % 3 - 1
                base = W2 * (1 + dy) + dx + 1
                acc9 += l2[t9].T @ h1[:, base:base + H2_F]
            h2[:, 1:1 + H2_F] = np.maximum(acc9 + b2s, 0.0)
            h2 = h2.astype(f16).astype(np.float32)
            h2v = h2[:, 1:1 + H2_F].reshape(128, H2_ROWS, W2)
            h2v[:, :, 0] = 0.0
            h2v[:, :, W2 - 1] = 0.0
            if t_i == 0:
                h2[0:32, 1:1 + W2] = 0.0
            if t_i == NT - 1:
                h2[96:128, 1 + (H2_ROWS - 1) * W2:1 + H2_F] = 0.0
            acc9 = np.zeros((4, O_F), np.float32)
            for t9 in range(9):
                dy, dx = t9 // 3 - 1, t9 % 3 - 1
                base = W2 * (1 + dy) + dx + 1
                acc9 += l3[t9].T @ h2[:, base:base + O_F]
            o[st_i] = acc9 + b3s
        outs.append({"o": o.astype(f16)})
    return outs


def _run(in_maps):
    if EMULATE:
        return _emulate(in_maps)
    if "nc" not in _CACHE:
        _apply_tile_patch()
        _CACHE["nc"] = _build()
    from concourse.bass_utils import run_bass_kernel_spmd
    import time as _time
    t0 = _time.time()
    res = run_bass_kernel_spmd(
        _CACHE["nc"], in_maps, core_ids=list(range(NCORES)),
    )
    _CACHE.setdefault("wall_ns", {})["fused"] = int((_time.time() - t0) * 1e9)
    if res.exec_time_ns is not None:
        _CACHE.setdefault("exec_ns", {})["fused"] = res.exec_time_ns
    return res.results


# ---------------------------------------------------------------- main entry
def kernel(sv_uncal, sv_bg, kernel, w1, b1, w2, b2, w3, b3, msk_idx, row_idx):
    sv_uncal = np.asarray(sv_uncal, np.float32)
    sv_bg = np.asarray(sv_bg, np.float32)
    w1 = np.asarray(w1, np.float32)
    b1 = np.asarray(b1, np.float32)
    w2 = np.asarray(w2, np.float32)
    b2 = np.asarray(b2, np.float32)
    w3 = np.asarray(w3, np.float32)
    b3 = np.asarray(b3, np.float32)
    msk_idx = np.asarray(msk_idx)
    row_idx = np.asarray(row_idx)

    # ---- host gather + replicate pad (zero tail to NPAD rows)
    fy = sv_uncal.reshape(B * P, H, W)[msk_idx][:, row_idx]   # [24, 1100, 52]
    fs = sv_bg.reshape(B * P, H, W)[msk_idx][:, row_idx]
    fyp = np.zeros((M_SEL, NPAD, W), np.float32)
    fsp = np.zeros((M_SEL, NPAD, W), np.float32)
    fyp[:, HALF:HALF + HI] = fy
    fsp[:, HALF:HALF + HI] = fs
    fyp[:, :HALF] = fy[:, 0:1]
    fsp[:, :HALF] = fs[:, 0:1]
    fyp[:, HALF + HI:HALF + HI + HALF] = fy[:, -1:]
    fsp[:, HALF + HI:HALF + HI + HALF] = fs[:, -1:]

    bands = _bands_from_kernel(kernel)
    bands16 = np.ascontiguousarray(bands.T[:, :, None]).astype(np.float16)
    w1f = np.concatenate(
        [w1[:, 0:10] + w1[:, 11:21], w1[:, 10:11], w1[:, 21:22]], axis=1)
    w1t = np.stack([w1f[:, :, t9 // 3, t9 % 3].T for t9 in range(9)])
    w2t = np.stack([w2[:, :, t9 // 3, t9 % 3].T for t9 in range(9)])
    w3t = np.stack([w3[0, :, t9 // 3, t9 % 3][:, None] for t9 in range(9)])
    shared = dict(
        bands=bands16,
        w1t=w1t.astype(np.float16),
        w2t=w2t.astype(np.float16),
        w3t=w3t.astype(np.float16),
        b1d=b1[:, None].astype(np.float32),
        b2d=b2[:, None].astype(np.float32),
        b3d=np.full((1, 1), b3[0] + np.float32(NS[0] / NS[1]), np.float32),
    )
    in_maps = []
    for c in range(NCORES):
        sl = slice(SW * c, SW * c + SW)
        in_maps.append(dict(
            fyp=np.ascontiguousarray(
                fyp[sl].transpose(1, 0, 2).reshape(NPAD, NF)).astype(np.float16),
            fsp=np.ascontiguousarray(
                fsp[sl].transpose(1, 0, 2).reshape(NPAD, NF)).astype(np.float16),
            **shared))

    res = _run(in_maps)

    # ---- assemble + fs + scatter (host)
    outs = []
    for c in range(NCORES):
        oo = res[c]["o"].astype(np.float32)
        oo = oo.reshape(SW, NT, 4, R, W2)[:, :, :, :, 1:1 + W]
        outs.append(oo.transpose(0, 2, 1, 3, 4).reshape(SW, HI, W))
    o_dev = np.concatenate(outs, axis=0)                      # [24, 1100, 52]
    out = o_dev + fs

    out_cal = np.zeros((B * P, HI, W), np.float32)
    np.add.at(out_cal, msk_idx, out)
    cnt = np.zeros((B * P,), np.float32)
    np.add.at(cnt, msk_idx, 1.0)
    out_msk = np.broadcast_to(
        (cnt > 0)[:, None, None], (B * P, HI, W)).copy()
    return (out_cal.reshape(B, P, HI, W),
            out_msk.reshape(B, P, HI, W))


# revision 9
# speedup vs baseline: 5.4965x; 5.4965x over previous
"""Trainium2 Bass kernel for nn_CalibrationModelObsGridGeometry.

Single fused SPMD launch on 8 cores (3 swaths/core), minimal host<->device
traffic (the axon tunnel is ~10 MB/s, so bytes moved dominate wall time):

  host:   gather + replicate-pad fy/fs, cast fp16               (~0.75 MB/core)
  device: window via DMA -> gaussian-pyramid Toeplitz matmuls (fp16) ->
          per-core BN partial sums -> 8-core AllReduce (96 B) ->
          fold stats into a per-partition scale/bias activation ->
          3x3 conv stack as accumulating matmuls (fp16, block-diag over
          4 row-quarters) -> fp16 output                        (~0.36 MB/core)
  host:   + fs_sel + scatter-add, mask.

Toeplitz bands, block-diagonal conv weights and biases are assembled
on-device from tiny uploads instead of shipping the expanded forms.
"""

import numpy as np

# ---------------------------------------------------------------- constants
B, P, H, W = 4, 8, 1200, 52
M_SEL, HI = 24, 1100
SIZE = 75
HALF = SIZE // 2  # 37
NS = (0.31446309894037083, 0.3886609494201447)
BN_EPS = 1e-5
HID = 32
NCORES = 8
SW = 3                      # swaths per core
NWIN = 21                   # toeplitz windows per swath (54 out rows each)
WJ = 54                     # out rows per window
NPAD = WJ * (NWIN - 1) + 128  # 1208 padded input rows
NQ = 4                      # h-quarters (partition groups)
QROWS = HI // NQ            # 275
NT = 5                      # processing tiles per swath
R = QROWS // NT             # 55 out rows per tile per quarter
W2 = 54                     # padded width
CAL_ROWS = R + 6            # 61 stored cal rows per tile
H1_ROWS = R + 4             # 59
H2_ROWS = R + 2             # 57
CAL_F = CAL_ROWS * W2       # 3294
H1_F = H1_ROWS * W2         # 3186
H2_F = H2_ROWS * W2         # 3078
O_F = R * W2                # 2970
CAL_SZ = CAL_F + 2          # +1 lead, +1 tail guard
H1_SZ = H1_F + 2
H2_SZ = H2_F + 2
CHUNK = 486                 # <=512 fp32 psum-bank limit
NST = SW * NT               # 15 processing tiles per core
NF = SW * W                 # 156
INVN = 1.0 / float(M_SEL * HI * W)
SCRROWS = 3 + WJ * NWIN + 3  # 1140 cal scratch rows (3 lead, tail garbage)

EMULATE = False             # numpy-emulate the device kernel (debug)

# single packed fp16 input blob (per core): fy, fs, bands, w1t, w2t, w3t,
# b1/b2/b3 (f32 bit-cast to f16 pairs)
OFF_FY = 0
OFF_FS = OFF_FY + NPAD * NF
OFF_BANDS = OFF_FS + NPAD * NF
OFF_W1 = OFF_BANDS + SIZE * 12
OFF_W2 = OFF_W1 + 9 * 12 * HID
OFF_W3 = OFF_W2 + 9 * HID * HID
OFF_B1 = OFF_W3 + 9 * HID
OFF_B2 = OFF_B1 + 2 * HID
OFF_B3 = OFF_B2 + 2 * HID
NBLOB = OFF_B3 + 2


def _bands_from_kernel(kern):
    """12 cal channels as 75-tap bands: D0..D9, A(=G9 on fy), B(=G9 on fs)."""
    g = np.asarray(kern, np.float32).reshape(10, SIZE)
    bands = np.zeros((12, SIZE), np.float32)
    bands[0] = -g[0]
    bands[0, HALF] += 1.0
    for i in range(1, 10):
        bands[i] = g[i - 1] - g[i]
    bands[10] = g[9]
    bands[11] = g[9]
    return bands


def _chunks(total):
    out = []
    off = 0
    while off < total:
        sz = min(CHUNK, total - off)
        out.append((off, sz))
        off += sz
    return out


# ---------------------------------------------------------------- device build
_CACHE = {}


def _apply_tile_patch():
    import concourse.tile as tile
    from concourse import mybir
    from concourse.vector_clock import ScopedClock

    def _patched(self, tick_clock, wait_clock):
        nc = self.nc
        drain_inst = nc.sync.drain()
        wait_clock.add_sem_waits(
            drain_inst.ins, ScopedClock({None: tick_clock.global_clock})
        )
        si = drain_inst.ins.sync_info
        if si is not None and si.on_wait and len(si.on_wait) > 1:
            extra = list(si.on_wait[1:])
            del si.on_wait[1:]
            for w in extra:
                d2 = nc.sync.drain()
                si2 = d2.ins.sync_info
                if si2 is None:
                    d2.ins.sync_info = mybir.SyncInfo(on_wait=[w], on_update=[])
                else:
                    si2.on_wait.append(w)
        nc.all_engine_barrier()
        popped = nc._tile_sem_poison_stack.pop()
        assert popped is self._sem_poison
        nc.clear_and_free_semaphores(list(self.sems.allocated().values()))
        nc.all_engine_barrier()

    tile.TileContext._drain_and_barrier = _patched


_WSPLIT_N = [0]


def _split_waits(nc):
    """This walrus build accepts only one sync-wait per instruction: hoist
    extra waits onto same-engine NoOps placed just before the instruction."""
    from concourse import mybir
    for f in nc.m.functions:
        for bb in f.blocks:
            new_list = []
            for ins in bb.instructions:
                si = getattr(ins, "sync_info", None)
                if si is not None and si.on_wait and len(si.on_wait) > 1:
                    extra = list(si.on_wait[:-1])
                    del si.on_wait[:-1]
                    for w in extra:
                        _WSPLIT_N[0] += 1
                        nop = mybir.InstDrain(
                            name=f"WSPLIT-{_WSPLIT_N[0]}",
                            engine=ins.engine,
                            sync_info=mybir.SyncInfo(on_wait=[w], on_update=[]),
                            bass_is_fusable=False,
                        )
                        new_list.append(nop)
                new_list.append(ins)
            bb.instructions[:] = new_list


def _build():
    import concourse.bass as bass
    import concourse.tile as tile
    from concourse import mybir

    f32 = mybir.dt.float32
    f16 = mybir.dt.float16
    nc = bass.Bass("TRN2", num_devices=NCORES)
    blob = nc.dram_tensor("blob", [NBLOB], f16, kind="ExternalInput")
    o = nc.dram_tensor("o", [NST, 4, O_F], f16, kind="ExternalOutput")

    Relu = mybir.ActivationFunctionType.Relu
    Ident = mybir.ActivationFunctionType.Identity
    Square = mybir.ActivationFunctionType.Square
    Sqrt = mybir.ActivationFunctionType.Sqrt
    ADD = mybir.AluOpType.add
    MULT = mybir.AluOpType.mult
    SUB = mybir.AluOpType.subtract
    AXX = mybir.AxisListType.X

    with tile.TileContext(nc) as tc:
        with (
            tc.tile_pool(name="singles", bufs=1) as singles,
            tc.tile_pool(name="dram", bufs=1, space="DRAM") as dram,
        ):
            # ---------------- setup: windows, toeplitz, weights
            fyw_s = singles.tile([128, NWIN, NF], f16)
            fsw_s = singles.tile([128, NWIN, NF], f16)
            for w in range(NWIN):
                nc.sync.dma_start(
                    out=fyw_s[:, w, :],
                    in_=blob[OFF_FY + NF * WJ * w:OFF_FY + NF * (WJ * w + 128)]
                    .rearrange("(r c) -> r c", c=NF))
                nc.sync.dma_start(
                    out=fsw_s[:, w, :],
                    in_=blob[OFF_FS + NF * WJ * w:OFF_FS + NF * (WJ * w + 128)]
                    .rearrange("(r c) -> r c", c=NF))
            toep_s = singles.tile([128, 12, WJ], f16)
            nc.vector.memset(toep_s[:], 0.0)
            bands_ap = blob[OFF_BANDS:OFF_BANDS + SIZE * 12].rearrange(
                "(d ch one) -> d ch one", ch=12, one=1)
            for j in range(WJ):
                nc.sync.dma_start(out=toep_s[j:j + SIZE, :, j:j + 1], in_=bands_ap)
            l1s = singles.tile([48, 9, 128], f16)
            l2s = singles.tile([128, 9, 128], f16)
            l3s = singles.tile([128, 9, 4], f16)
            nc.vector.memset(l1s[:], 0.0)
            nc.vector.memset(l2s[:], 0.0)
            nc.vector.memset(l3s[:], 0.0)
            for t9 in range(9):
                w1_ap = blob[OFF_W1 + 12 * HID * t9:OFF_W1 + 12 * HID * (t9 + 1)
                             ].rearrange("(a b) -> a b", b=HID)
                w2_ap = blob[OFF_W2 + HID * HID * t9:OFF_W2 + HID * HID * (t9 + 1)
                             ].rearrange("(a b) -> a b", b=HID)
                w3_ap = blob[OFF_W3 + HID * t9:OFF_W3 + HID * (t9 + 1)
                             ].rearrange("(a b) -> a b", b=1)
                for q in range(NQ):
                    nc.sync.dma_start(
                        out=l1s[12 * q:12 * q + 12, t9, 32 * q:32 * q + 32],
                        in_=w1_ap)
                    nc.sync.dma_start(
                        out=l2s[32 * q:32 * q + 32, t9, 32 * q:32 * q + 32],
                        in_=w2_ap)
                    nc.sync.dma_start(
                        out=l3s[32 * q:32 * q + 32, t9, q:q + 1],
                        in_=w3_ap)
            b1s = singles.tile([128, 1], f32)
            b2s = singles.tile([128, 1], f32)
            b3s = singles.tile([4, 1], f32)
            b1_ap = blob[OFF_B1:OFF_B1 + 2 * HID].rearrange(
                "(a b) -> a b", b=2).bitcast(f32)
            b2_ap = blob[OFF_B2:OFF_B2 + 2 * HID].rearrange(
                "(a b) -> a b", b=2).bitcast(f32)
            b3_ap = blob[OFF_B3:OFF_B3 + 2].rearrange(
                "(a b) -> a b", b=2).bitcast(f32)
            for q in range(NQ):
                nc.sync.dma_start(out=b1s[32 * q:32 * q + 32, :], in_=b1_ap)
                nc.sync.dma_start(out=b2s[32 * q:32 * q + 32, :], in_=b2_ap)
                nc.sync.dma_start(out=b3s[q:q + 1, :], in_=b3_ap)
            ones54 = singles.tile([WJ, 1], f32)
            nc.vector.memset(ones54[:], 1.0)
            acc = singles.tile([WJ, 24], f32)
            nc.vector.memset(acc[:], 0.0)
            zrow = singles.tile([12, 3 * W2], f16)
            nc.vector.memset(zrow[:], 0.0)

            cal_d = dram.tile([12, SCRROWS, SW, W2], f16)
            stat_in = dram.tile([1, 24], f32)
            stat_out = dram.tile([1, 24], f32)
            mr_rd = dram.tile([NQ, 12, 1], f32)
            mr_nd = dram.tile([NQ, 12, 1], f32)

            # ---------------- phase A: gaussian pyramid + BN partial stats
            with (
                tc.tile_pool(name="stage", bufs=2) as stage,
                tc.tile_pool(name="psumA", bufs=4, space="PSUM") as psumA,
            ):
                for w in range(NWIN):
                    st = stage.tile([WJ, 12, NF], f16, tag="st")
                    for ch in range(12):
                        src = fsw_s if ch == 11 else fyw_s
                        ps = psumA.tile([WJ, NF], f32, tag="psA")
                        nc.tensor.matmul(
                            ps[:], lhsT=toep_s[:, ch, :], rhs=src[:, w, :],
                            start=True, stop=True)
                        nc.scalar.copy(st[:, ch, :], ps[:])
                    for s in range(SW):
                        nc.sync.dma_start(
                            out=cal_d[:, 3 + WJ * w:3 + WJ * w + WJ, s, 1:1 + W]
                            .rearrange("ch r c -> r ch c"),
                            in_=st[:, :, W * s:W * s + W])
                    vr = WJ if w < NWIN - 1 else HI - WJ * (NWIN - 1)  # 54 / 20
                    red1 = stage.tile([WJ, 12], f32, tag="red1")
                    nc.vector.tensor_reduce(
                        out=red1[0:vr], in_=st[0:vr], axis=AXX, op=ADD)
                    sq = stage.tile([WJ, 12, NF], f16, tag="sq")
                    nc.scalar.activation(out=sq[0:vr], in_=st[0:vr], func=Square)
                    red2 = stage.tile([WJ, 12], f32, tag="red2")
                    nc.vector.tensor_reduce(
                        out=red2[0:vr], in_=sq[0:vr], axis=AXX, op=ADD)
                    nc.vector.tensor_tensor(
                        out=acc[0:vr, 0:12], in0=acc[0:vr, 0:12],
                        in1=red1[0:vr], op=ADD)
                    nc.vector.tensor_tensor(
                        out=acc[0:vr, 12:24], in0=acc[0:vr, 12:24],
                        in1=red2[0:vr], op=ADD)

                # ---------------- BN stats: partition-sum, allreduce, fold
                ps1 = psumA.tile([1, 24], f32, tag="ps1")
                nc.tensor.matmul(ps1[:], lhsT=ones54[:], rhs=acc[:],
                                 start=True, stop=True)
                stat_sb = singles.tile([1, 24], f32)
                nc.scalar.copy(stat_sb[:], ps1[:])
            import os as _os
            _nocc = bool(_os.environ.get("NOCC"))
            nc.gpsimd.dma_start(stat_in[:], stat_sb[:])
            if _nocc:
                nc.gpsimd.dma_start(stat_out[:], stat_in[:])
            else:
                nc.gpsimd.collective_compute(
                    "AllReduce", ADD, replica_groups=[list(range(NCORES))],
                    ins=[stat_in.opt()], outs=[stat_out.opt()])
            gl = singles.tile([1, 24], f32)
            nc.gpsimd.dma_start(gl[:], stat_out[:])
            _invn = INVN * (NCORES if _nocc else 1)
            mt = singles.tile([1, 12], f32)
            nc.vector.tensor_scalar_mul(mt[:], gl[:, 0:12], _invn)
            var = singles.tile([1, 12], f32)
            nc.vector.tensor_tensor(out=var[:], in0=mt[:], in1=mt[:], op=MULT)
            e2 = singles.tile([1, 12], f32)
            nc.vector.tensor_scalar_mul(e2[:], gl[:, 12:24], _invn)
            nc.vector.tensor_tensor(out=var[:], in0=e2[:], in1=var[:], op=SUB)
            eps_t = singles.tile([1, 1], f32)
            nc.vector.memset(eps_t[:], BN_EPS)
            sd = singles.tile([1, 12], f32)
            nc.scalar.activation(out=sd[:], in_=var[:], func=Sqrt,
                                 bias=eps_t[:, 0:1])
            rr_t = singles.tile([1, 12], f32)
            nc.vector.reciprocal(rr_t[:], sd[:])
            nmr = singles.tile([1, 12], f32)
            nc.vector.tensor_tensor(out=nmr[:], in0=mt[:], in1=rr_t[:], op=MULT)
            nc.vector.tensor_scalar_mul(nmr[:], nmr[:], -1.0)
            for q in range(NQ):
                nc.sync.dma_start(out=mr_rd[q:q + 1, :, 0], in_=rr_t[0:1, :])
                nc.sync.dma_start(out=mr_nd[q:q + 1, :, 0], in_=nmr[0:1, :])
            r48 = singles.tile([48, 1], f32)
            nc.sync.dma_start(
                out=r48[:], in_=mr_rd[:].rearrange("q c one -> (q c) one"))
            nmr48 = singles.tile([48, 1], f32)
            nc.sync.dma_start(
                out=nmr48[:], in_=mr_nd[:].rearrange("q c one -> (q c) one"))

            # ---------------- phase B: normalize + 3x3 conv stack
            with (
                tc.tile_pool(name="io", bufs=2) as io,
                tc.tile_pool(name="acts", bufs=2) as acts,
                tc.tile_pool(name="psum", bufs=6, space="PSUM") as psum,
                tc.tile_pool(name="psum3", bufs=2, space="PSUM") as psum3,
            ):
                for st_i in range(NST):
                    s_i, t_i = divmod(st_i, NT)
                    calr = io.tile([48, CAL_SZ], f16, tag="calr")
                    for q in range(NQ):
                        r0 = R * t_i + QROWS * q
                        nc.sync.dma_start(
                            out=calr[12 * q:12 * q + 12, 1:1 + CAL_F]
                            .rearrange("p (r c) -> p r c", c=W2),
                            in_=cal_d[:, r0:r0 + CAL_ROWS, s_i, :])
                    caln = io.tile([48, CAL_SZ], f16, tag="caln")
                    nc.scalar.activation(
                        out=caln[:], in_=calr[:], func=Ident,
                        scale=r48[:, 0:1], bias=nmr48[:, 0:1])
                    cv = caln[:, 1:1 + CAL_F].rearrange("p (r c) -> p r c", c=W2)
                    nc.vector.memset(cv[:, :, 0:1], 0.0)
                    nc.vector.memset(cv[:, :, W2 - 1:W2], 0.0)
                    nc.vector.memset(caln[:, 0:1], 0.0)
                    nc.vector.memset(caln[:, 1 + CAL_F:], 0.0)
                    if t_i == 0:      # swath top: zero pad rows of quarter 0
                        nc.vector.memset(caln[0:12, 1:1 + 3 * W2], 0.0)
                    if t_i == NT - 1:  # swath bottom: zero pad rows of quarter 3
                        # (DMA: compute-engine APs need 32-aligned partition start)
                        nc.sync.dma_start(
                            out=caln[36:48, 1 + (CAL_ROWS - 3) * W2:1 + CAL_F],
                            in_=zrow[:])

                    h1 = acts.tile([128, H1_SZ], f16, tag="h1")
                    h2 = acts.tile([128, H2_SZ], f16, tag="h2")
                    ot = io.tile([4, O_F], f16, tag="ot")

                    # ---- conv1: caln[48] -> h1[128], ReLU(. + b1)
                    for off, sz in _chunks(H1_F):
                        ps = psum.tile([128, CHUNK], f32, tag="ps")
                        for t9 in range(9):
                            dy, dx = t9 // 3 - 1, t9 % 3 - 1
                            base = off + W2 * (1 + dy) + dx + 1
                            nc.tensor.matmul(
                                ps[:, :sz], lhsT=l1s[:, t9, :],
                                rhs=caln[:, base:base + sz],
                                start=(t9 == 0), stop=(t9 == 8),
                            )
                        nc.scalar.activation(
                            out=h1[:, 1 + off:1 + off + sz], in_=ps[:, :sz],
                            func=Relu, bias=b1s[:, 0:1], scale=1.0,
                        )
                    h1v = h1[:, 1:1 + H1_F].rearrange("p (r c) -> p r c", c=W2)
                    nc.vector.memset(h1v[:, :, 0:1], 0.0)
                    nc.vector.memset(h1v[:, :, W2 - 1:W2], 0.0)
                    if t_i == 0:
                        nc.vector.memset(h1[0:32, 1:1 + 2 * W2], 0.0)
                    if t_i == NT - 1:
                        nc.vector.memset(
                            h1[96:128, 1 + (H1_ROWS - 2) * W2:1 + H1_F], 0.0)

                    # ---- conv2: h1[128] -> h2[128], ReLU(. + b2)
                    for off, sz in _chunks(H2_F):
                        ps = psum.tile([128, CHUNK], f32, tag="ps")
                        for t9 in range(9):
                            dy, dx = t9 // 3 - 1, t9 % 3 - 1
                            base = off + W2 * (1 + dy) + dx + 1
                            nc.tensor.matmul(
                                ps[:, :sz], lhsT=l2s[:, t9, :],
                                rhs=h1[:, base:base + sz],
                                start=(t9 == 0), stop=(t9 == 8),
                            )
                        nc.scalar.activation(
                            out=h2[:, 1 + off:1 + off + sz], in_=ps[:, :sz],
                            func=Relu, bias=b2s[:, 0:1], scale=1.0,
                        )
                    h2v = h2[:, 1:1 + H2_F].rearrange("p (r c) -> p r c", c=W2)
                    nc.vector.memset(h2v[:, :, 0:1], 0.0)
                    nc.vector.memset(h2v[:, :, W2 - 1:W2], 0.0)
                    if t_i == 0:
                        nc.vector.memset(h2[0:32, 1:1 + W2], 0.0)
                    if t_i == NT - 1:
                        nc.vector.memset(
                            h2[96:128, 1 + (H2_ROWS - 1) * W2:1 + H2_F], 0.0)

                    # ---- conv3: h2[128] -> o[4], Identity(. + b3')
                    for off, sz in _chunks(O_F):
                        ps = psum3.tile([4, CHUNK], f32, tag="ps3")
                        for t9 in range(9):
                            dy, dx = t9 // 3 - 1, t9 % 3 - 1
                            base = off + W2 * (1 + dy) + dx + 1
                            nc.tensor.matmul(
                                ps[:, :sz], lhsT=l3s[:, t9, :],
                                rhs=h2[:, base:base + sz],
                                start=(t9 == 0), stop=(t9 == 8),
                            )
                        nc.scalar.activation(
                            out=ot[:, off:off + sz], in_=ps[:, :sz],
                            func=Ident, bias=b3s[:, 0:1], scale=1.0,
                        )
                    nc.sync.dma_start(out=o[st_i], in_=ot[:])
    _split_waits(nc)
    return nc


# ---------------------------------------------------------------- emulation
def _emulate(in_maps):
    """Numpy mirror of the fused device kernel (fp16 casts at tile edges)."""
    f16 = np.float16
    stats = np.zeros((1, 24), np.float32)
    cores = []
    for m in in_maps:
        blob = m["blob"]
        fyp = blob[OFF_FY:OFF_FS].astype(np.float32).reshape(NPAD, NF)
        fsp = blob[OFF_FS:OFF_BANDS].astype(np.float32).reshape(NPAD, NF)
        bands = blob[OFF_BANDS:OFF_W1].astype(np.float32).reshape(SIZE, 12)
        toep = np.zeros((128, 12, WJ), np.float32)
        for j in range(WJ):
            toep[j:j + SIZE, :, j] = bands
        cal_d = np.zeros((12, SCRROWS, SW, W2), np.float32)
        acc = np.zeros((WJ, 24), np.float32)
        for w in range(NWIN):
            st = np.zeros((WJ, 12, NF), np.float32)
            for ch in range(12):
                src = fsp if ch == 11 else fyp
                st[:, ch, :] = toep[:, ch, :].T @ src[WJ * w:WJ * w + 128, :]
            st = st.astype(f16).astype(np.float32)
            cal_d[:, 3 + WJ * w:3 + WJ * w + WJ, :, 1:1 + W] = (
                st.reshape(WJ, 12, SW, W).transpose(1, 0, 2, 3))
            vr = WJ if w < NWIN - 1 else HI - WJ * (NWIN - 1)
            acc[0:vr, 0:12] += st[0:vr].sum(2)
            sq = (st[0:vr] ** 2).astype(f16).astype(np.float32)
            acc[0:vr, 12:24] += sq.sum(2)
        stats += acc.sum(0, keepdims=True)
        cores.append(cal_d)
    mt = stats[:, 0:12] * INVN
    var = stats[:, 12:24] * INVN - mt * mt
    rr = 1.0 / np.sqrt(var + BN_EPS)
    nmr = -(mt * rr)
    r48 = np.tile(rr[0], NQ)[:, None]
    nmr48 = np.tile(nmr[0], NQ)[:, None]

    outs = []
    for m, cal_d in zip(in_maps, cores):
        l1 = np.zeros((9, 48, 128), np.float32)
        l2 = np.zeros((9, 128, 128), np.float32)
        l3 = np.zeros((9, 128, 4), np.float32)
        blob = m["blob"]
        w1t = blob[OFF_W1:OFF_W2].astype(np.float32).reshape(9, 12, HID)
        w2t = blob[OFF_W2:OFF_W3].astype(np.float32).reshape(9, HID, HID)
        w3t = blob[OFF_W3:OFF_B1].astype(np.float32).reshape(9, HID, 1)
        b1v = blob[OFF_B1:OFF_B2].view(np.float32)
        b2v = blob[OFF_B2:OFF_B3].view(np.float32)
        b3v = blob[OFF_B3:OFF_B3 + 2].view(np.float32)
        for t9 in range(9):
            for q in range(NQ):
                l1[t9, 12 * q:12 * q + 12, 32 * q:32 * q + 32] = w1t[t9]
                l2[t9, 32 * q:32 * q + 32, 32 * q:32 * q + 32] = w2t[t9]
                l3[t9, 32 * q:32 * q + 32, q] = w3t[t9][:, 0]
        b1s = np.tile(b1v, NQ)[:, None]
        b2s = np.tile(b2v, NQ)[:, None]
        b3s = np.full((4, 1), b3v[0], np.float32)
        o = np.zeros((NST, 4, O_F), np.float32)
        for st_i in range(NST):
            s_i, t_i = divmod(st_i, NT)
            calr = np.zeros((48, CAL_SZ), np.float32)
            for q in range(NQ):
                r0 = R * t_i + QROWS * q
                calr[12 * q:12 * q + 12, 1:1 + CAL_F] = (
                    cal_d[:, r0:r0 + CAL_ROWS, s_i, :].reshape(12, CAL_F))
            caln = (calr * r48 + nmr48).astype(f16).astype(np.float32)
            cv = caln[:, 1:1 + CAL_F].reshape(48, CAL_ROWS, W2)
            cv[:, :, 0] = 0.0
            cv[:, :, W2 - 1] = 0.0
            caln[:, 0] = 0.0
            caln[:, 1 + CAL_F:] = 0.0
            if t_i == 0:
                caln[0:12, 1:1 + 3 * W2] = 0.0
            if t_i == NT - 1:
                caln[36:48, 1 + (CAL_ROWS - 3) * W2:1 + CAL_F] = 0.0
            h1 = np.zeros((128, H1_SZ), np.float32)
            acc9 = np.zeros((128, H1_F), np.float32)
            for t9 in range(9):
                dy, dx = t9 // 3 - 1, t9 % 3 - 1
                base = W2 * (1 + dy) + dx + 1
                acc9 += l1[t9].T @ caln[:, base:base + H1_F]
            h1[:, 1:1 + H1_F] = np.maximum(acc9 + b1s, 0.0)
            h1 = h1.astype(f16).astype(np.float32)
            h1v = h1[:, 1:1 + H1_F].reshape(128, H1_ROWS, W2)
            h1v[:, :, 0] = 0.0
            h1v[:, :, W2 - 1] = 0.0
            if t_i == 0:
                h1[0:32, 1:1 + 2 * W2] = 0.0
            if t_i == NT - 1:
                h1[96:128, 1 + (H1_ROWS - 2) * W2:1 + H1_F] = 0.0
            h2 = np.zeros((128, H2_SZ), np.float32)
            acc9 = np.zeros((128, H2_F), np.float32)
            for t9 in range(9):
                dy, dx = t9 // 3 - 1, t9 % 3 - 1
                base = W2 * (1 + dy) + dx + 1
                acc9 += l2[t9].T @ h1[:, base:base + H2_F]
            h2[:, 1:1 + H2_F] = np.maximum(acc9 + b2s, 0.0)
            h2 = h2.astype(f16).astype(np.float32)
            h2v = h2[:, 1:1 + H2_F].reshape(128, H2_ROWS, W2)
            h2v[:, :, 0] = 0.0
            h2v[:, :, W2 - 1] = 0.0
            if t_i == 0:
                h2[0:32, 1:1 + W2] = 0.0
            if t_i == NT - 1:
                h2[96:128, 1 + (H2_ROWS - 1) * W2:1 + H2_F] = 0.0
            acc9 = np.zeros((4, O_F), np.float32)
            for t9 in range(9):
                dy, dx = t9 // 3 - 1, t9 % 3 - 1
                base = W2 * (1 + dy) + dx + 1
                acc9 += l3[t9].T @ h2[:, base:base + O_F]
            o[st_i] = acc9 + b3s
        outs.append({"o": o.astype(f16)})
    return outs


def _run(in_maps):
    if EMULATE:
        return _emulate(in_maps)
    if "nc" not in _CACHE:
        _apply_tile_patch()
        _CACHE["nc"] = _build()
    from concourse.bass_utils import run_bass_kernel_spmd
    import time as _time
    t0 = _time.time()
    res = run_bass_kernel_spmd(
        _CACHE["nc"], in_maps, core_ids=list(range(NCORES)),
    )
    _CACHE.setdefault("wall_ns", {})["fused"] = int((_time.time() - t0) * 1e9)
    if res.exec_time_ns is not None:
        _CACHE.setdefault("exec_ns", {})["fused"] = res.exec_time_ns
    return res.results


# ---------------------------------------------------------------- main entry
def kernel(sv_uncal, sv_bg, kernel, w1, b1, w2, b2, w3, b3, msk_idx, row_idx):
    sv_uncal = np.asarray(sv_uncal, np.float32)
    sv_bg = np.asarray(sv_bg, np.float32)
    w1 = np.asarray(w1, np.float32)
    b1 = np.asarray(b1, np.float32)
    w2 = np.asarray(w2, np.float32)
    b2 = np.asarray(b2, np.float32)
    w3 = np.asarray(w3, np.float32)
    b3 = np.asarray(b3, np.float32)
    msk_idx = np.asarray(msk_idx)
    row_idx = np.asarray(row_idx)

    # ---- host gather + replicate pad (zero tail to NPAD rows)
    fy = sv_uncal.reshape(B * P, H, W)[msk_idx][:, row_idx]   # [24, 1100, 52]
    fs = sv_bg.reshape(B * P, H, W)[msk_idx][:, row_idx]
    fyp = np.zeros((M_SEL, NPAD, W), np.float32)
    fsp = np.zeros((M_SEL, NPAD, W), np.float32)
    fyp[:, HALF:HALF + HI] = fy
    fsp[:, HALF:HALF + HI] = fs
    fyp[:, :HALF] = fy[:, 0:1]
    fsp[:, :HALF] = fs[:, 0:1]
    fyp[:, HALF + HI:HALF + HI + HALF] = fy[:, -1:]
    fsp[:, HALF + HI:HALF + HI + HALF] = fs[:, -1:]

    bands = _bands_from_kernel(kernel)
    bands16 = np.ascontiguousarray(bands.T[:, :, None]).astype(np.float16)
    w1f = np.concatenate(
        [w1[:, 0:10] + w1[:, 11:21], w1[:, 10:11], w1[:, 21:22]], axis=1)
    w1t = np.stack([w1f[:, :, t9 // 3, t9 % 3].T for t9 in range(9)])
    w2t = np.stack([w2[:, :, t9 // 3, t9 % 3].T for t9 in range(9)])
    w3t = np.stack([w3[0, :, t9 // 3, t9 % 3][:, None] for t9 in range(9)])
    wtail = np.empty(NBLOB - OFF_BANDS, np.float16)
    wtail[0:OFF_W1 - OFF_BANDS] = bands16.ravel()
    wtail[OFF_W1 - OFF_BANDS:OFF_W2 - OFF_BANDS] = (
        w1t.astype(np.float16).ravel())
    wtail[OFF_W2 - OFF_BANDS:OFF_W3 - OFF_BANDS] = (
        w2t.astype(np.float16).ravel())
    wtail[OFF_W3 - OFF_BANDS:OFF_B1 - OFF_BANDS] = (
        w3t.astype(np.float16).ravel())
    wtail[OFF_B1 - OFF_BANDS:OFF_B2 - OFF_BANDS] = (
        b1.astype(np.float32).view(np.float16))
    wtail[OFF_B2 - OFF_BANDS:OFF_B3 - OFF_BANDS] = (
        b2.astype(np.float32).view(np.float16))
    wtail[OFF_B3 - OFF_BANDS:] = np.asarray(
        [b3[0] + np.float32(NS[0] / NS[1])], np.float32).view(np.float16)
    in_maps = []
    for c in range(NCORES):
        sl = slice(SW * c, SW * c + SW)
        blob = np.empty(NBLOB, np.float16)
        blob[OFF_FY:OFF_FS] = np.ascontiguousarray(
            fyp[sl].transpose(1, 0, 2)).astype(np.float16).ravel()
        blob[OFF_FS:OFF_BANDS] = np.ascontiguousarray(
            fsp[sl].transpose(1, 0, 2)).astype(np.float16).ravel()
        blob[OFF_BANDS:] = wtail
        in_maps.append(dict(blob=blob))

    res = _run(in_maps)

    # ---- assemble + fs + scatter (host)
    outs = []
    for c in range(NCORES):
        oo = res[c]["o"].astype(np.float32)
        oo = oo.reshape(SW, NT, 4, R, W2)[:, :, :, :, 1:1 + W]
        outs.append(oo.transpose(0, 2, 1, 3, 4).reshape(SW, HI, W))
    o_dev = np.concatenate(outs, axis=0)                      # [24, 1100, 52]
    out = o_dev + fs

    out_cal = np.zeros((B * P, HI, W), np.float32)
    np.add.at(out_cal, msk_idx, out)
    cnt = np.zeros((B * P,), np.float32)
    np.add.at(cnt, msk_idx, 1.0)
    out_msk = np.broadcast_to(
        (cnt > 0)[:, None, None], (B * P, HI, W)).copy()
    return (out_cal.reshape(B, P, HI, W),
            out_msk.reshape(B, P, HI, W))


# revision 18
# speedup vs baseline: 71.4707x; 13.0029x over previous
"""Trainium2 Bass kernel for nn_CalibrationModelObsGridGeometry.

Single fused SPMD launch on 8 cores (3 swaths/core), one packed fp16 input
blob per core (~0.78 MB) and one fp16 output (~0.36 MB) to minimize
host<->device traffic and per-array transfer overhead:

  host:   gather + replicate-pad fy/fs, cast fp16, pack blob
  device: window via DMA -> gaussian-pyramid Toeplitz matmuls (fp16) ->
          per-core BN partial sums -> 8-core AllReduce (96 B) ->
          per-partition scale/bias normalization activation ->
          3x3 conv stack as accumulating matmuls (fp16, block-diag over
          4 row-quarters) -> fp16 output
  host:   + fs_sel + scatter-add, mask.

Toeplitz bands, block-diagonal conv weights and biases are assembled
on-device from the blob instead of shipping expanded forms.

Launch overhead is held down by (a) a /tmp disk cache of the built BIR so
warm processes skip Bass construction and tile scheduling (~1 s), (b) the
jax persistent compilation cache, and (c) debug-info normalization that
makes the BIR bytes reproducible regardless of build directory, so every
rebuild yields the identical NEFF and hits the runtime content cache.
"""

import numpy as np

# ---------------------------------------------------------------- constants
B, P, H, W = 4, 8, 1200, 52
M_SEL, HI = 24, 1100
SIZE = 75
HALF = SIZE // 2  # 37
NS = (0.31446309894037083, 0.3886609494201447)
BN_EPS = 1e-5
HID = 32
NCORES = 8
SW = 3                      # swaths per core
NWIN = 21                   # toeplitz windows per swath (54 out rows each)
WJ = 54                     # out rows per window
NPAD = WJ * (NWIN - 1) + 128  # 1208 padded input rows
NQ = 4                      # h-quarters (partition groups)
QROWS = HI // NQ            # 275
NT = 5                      # processing tiles per swath
R = QROWS // NT             # 55 out rows per tile per quarter
W2 = 54                     # padded width
CAL_ROWS = R + 6            # 61 stored cal rows per tile
H1_ROWS = R + 4             # 59
H2_ROWS = R + 2             # 57
CAL_F = CAL_ROWS * W2       # 3294
H1_F = H1_ROWS * W2         # 3186
H2_F = H2_ROWS * W2         # 3078
O_F = R * W2                # 2970
CAL_SZ = CAL_F + 2          # +1 lead, +1 tail guard
H1_SZ = H1_F + 2
H2_SZ = H2_F + 2
CHUNK = 486                 # <=512 fp32 psum-bank limit
NST = SW * NT               # 15 processing tiles per core
NF = SW * W                 # 156
INVN = 1.0 / float(M_SEL * HI * W)
SCRROWS = 3 + WJ * NWIN + 3  # 1140 cal scratch rows (3 lead, tail garbage)

EMULATE = False             # numpy-emulate the device kernel (debug)

# single packed fp16 input blob (per core): fy, fs, bands, w1t, w2t, w3t,
# b1/b2/b3 (f32 bit-cast to f16 pairs)
OFF_FY = 0
OFF_FS = OFF_FY + NPAD * NF
OFF_BANDS = OFF_FS + NPAD * NF
OFF_W1 = OFF_BANDS + SIZE * 12
OFF_W2 = OFF_W1 + 9 * 12 * HID
OFF_W3 = OFF_W2 + 9 * HID * HID
OFF_B1 = OFF_W3 + 9 * HID
OFF_B2 = OFF_B1 + 2 * HID
OFF_B3 = OFF_B2 + 2 * HID
NBLOB = OFF_B3 + 2


def _bands_from_kernel(kern):
    """12 cal channels as 75-tap bands: D0..D9, A(=G9 on fy), B(=G9 on fs)."""
    g = np.asarray(kern, np.float32).reshape(10, SIZE)
    bands = np.zeros((12, SIZE), np.float32)
    bands[0] = -g[0]
    bands[0, HALF] += 1.0
    for i in range(1, 10):
        bands[i] = g[i - 1] - g[i]
    bands[10] = g[9]
    bands[11] = g[9]
    return bands


def _chunks(total):
    out = []
    off = 0
    while off < total:
        sz = min(CHUNK, total - off)
        out.append((off, sz))
        off += sz
    return out


# ---------------------------------------------------------------- device build
_CACHE = {}


def _apply_tile_patch():
    import concourse.tile as tile
    from concourse import mybir
    from concourse.vector_clock import ScopedClock

    def _patched(self, tick_clock, wait_clock):
        nc = self.nc
        drain_inst = nc.sync.drain()
        wait_clock.add_sem_waits(
            drain_inst.ins, ScopedClock({None: tick_clock.global_clock})
        )
        si = drain_inst.ins.sync_info
        if si is not None and si.on_wait and len(si.on_wait) > 1:
            extra = list(si.on_wait[1:])
            del si.on_wait[1:]
            for w in extra:
                d2 = nc.sync.drain()
                si2 = d2.ins.sync_info
                if si2 is None:
                    d2.ins.sync_info = mybir.SyncInfo(on_wait=[w], on_update=[])
                else:
                    si2.on_wait.append(w)
        nc.all_engine_barrier()
        popped = nc._tile_sem_poison_stack.pop()
        assert popped is self._sem_poison
        nc.clear_and_free_semaphores(list(self.sems.allocated().values()))
        nc.all_engine_barrier()

    tile.TileContext._drain_and_barrier = _patched


_WSPLIT_N = [0]


def _split_waits(nc):
    """This walrus build accepts only one sync-wait per instruction: hoist
    extra waits onto same-engine NoOps placed just before the instruction."""
    from concourse import mybir
    for f in nc.m.functions:
        for bb in f.blocks:
            new_list = []
            for ins in bb.instructions:
                si = getattr(ins, "sync_info", None)
                if si is not None and si.on_wait and len(si.on_wait) > 1:
                    extra = list(si.on_wait[:-1])
                    del si.on_wait[:-1]
                    for w in extra:
                        _WSPLIT_N[0] += 1
                        nop = mybir.InstDrain(
                            name=f"WSPLIT-{_WSPLIT_N[0]}",
                            engine=ins.engine,
                            sync_info=mybir.SyncInfo(on_wait=[w], on_update=[]),
                            bass_is_fusable=False,
                        )
                        new_list.append(nop)
                new_list.append(ins)
            bb.instructions[:] = new_list


def _build():
    import concourse.bass as bass
    import concourse.tile as tile
    from concourse import mybir

    f32 = mybir.dt.float32
    f16 = mybir.dt.float16
    nc = bass.Bass("TRN2", num_devices=NCORES,
                   disable_frame_to_traceback=True)
    blob = nc.dram_tensor("blob", [NBLOB], f16, kind="ExternalInput")
    o = nc.dram_tensor("o", [NST, 4, O_F], f16, kind="ExternalOutput")

    Relu = mybir.ActivationFunctionType.Relu
    Ident = mybir.ActivationFunctionType.Identity
    Square = mybir.ActivationFunctionType.Square
    Sqrt = mybir.ActivationFunctionType.Sqrt
    ADD = mybir.AluOpType.add
    MULT = mybir.AluOpType.mult
    SUB = mybir.AluOpType.subtract
    AXX = mybir.AxisListType.X

    with tile.TileContext(nc) as tc:
        with (
            tc.tile_pool(name="singles", bufs=1) as singles,
            tc.tile_pool(name="dram", bufs=1, space="DRAM") as dram,
        ):
            # ---------------- setup: windows, toeplitz, weights
            fyw_s = singles.tile([128, NWIN, NF], f16)
            fsw_s = singles.tile([128, NWIN, NF], f16)
            for w in range(NWIN):
                nc.sync.dma_start(
                    out=fyw_s[:, w, :],
                    in_=blob[OFF_FY + NF * WJ * w:OFF_FY + NF * (WJ * w + 128)]
                    .rearrange("(r c) -> r c", c=NF))
                nc.sync.dma_start(
                    out=fsw_s[:, w, :],
                    in_=blob[OFF_FS + NF * WJ * w:OFF_FS + NF * (WJ * w + 128)]
                    .rearrange("(r c) -> r c", c=NF))
            toep_s = singles.tile([128, 12, WJ], f16)
            nc.vector.memset(toep_s[:], 0.0)
            bands_ap = blob[OFF_BANDS:OFF_BANDS + SIZE * 12].rearrange(
                "(d ch one) -> d ch one", ch=12, one=1)
            for j in range(WJ):
                nc.sync.dma_start(out=toep_s[j:j + SIZE, :, j:j + 1], in_=bands_ap)
            l1s = singles.tile([48, 9, 128], f16)
            l2s = singles.tile([128, 9, 128], f16)
            l3s = singles.tile([128, 9, 4], f16)
            nc.vector.memset(l1s[:], 0.0)
            nc.vector.memset(l2s[:], 0.0)
            nc.vector.memset(l3s[:], 0.0)
            for t9 in range(9):
                w1_ap = blob[OFF_W1 + 12 * HID * t9:OFF_W1 + 12 * HID * (t9 + 1)
                             ].rearrange("(a b) -> a b", b=HID)
                w2_ap = blob[OFF_W2 + HID * HID * t9:OFF_W2 + HID * HID * (t9 + 1)
                             ].rearrange("(a b) -> a b", b=HID)
                w3_ap = blob[OFF_W3 + HID * t9:OFF_W3 + HID * (t9 + 1)
                             ].rearrange("(a b) -> a b", b=1)
                for q in range(NQ):
                    nc.sync.dma_start(
                        out=l1s[12 * q:12 * q + 12, t9, 32 * q:32 * q + 32],
                        in_=w1_ap)
                    nc.sync.dma_start(
                        out=l2s[32 * q:32 * q + 32, t9, 32 * q:32 * q + 32],
                        in_=w2_ap)
                    nc.sync.dma_start(
                        out=l3s[32 * q:32 * q + 32, t9, q:q + 1],
                        in_=w3_ap)
            b1s = singles.tile([128, 1], f32)
            b2s = singles.tile([128, 1], f32)
            b3s = singles.tile([4, 1], f32)
            b1_ap = blob[OFF_B1:OFF_B1 + 2 * HID].rearrange(
                "(a b) -> a b", b=2).bitcast(f32)
            b2_ap = blob[OFF_B2:OFF_B2 + 2 * HID].rearrange(
                "(a b) -> a b", b=2).bitcast(f32)
            b3_ap = blob[OFF_B3:OFF_B3 + 2].rearrange(
                "(a b) -> a b", b=2).bitcast(f32)
            for q in range(NQ):
                nc.sync.dma_start(out=b1s[32 * q:32 * q + 32, :], in_=b1_ap)
                nc.sync.dma_start(out=b2s[32 * q:32 * q + 32, :], in_=b2_ap)
                nc.sync.dma_start(out=b3s[q:q + 1, :], in_=b3_ap)
            ones54 = singles.tile([WJ, 1], f32)
            nc.vector.memset(ones54[:], 1.0)
            acc = singles.tile([WJ, 24], f32)
            nc.vector.memset(acc[:], 0.0)
            zrow = singles.tile([12, 3 * W2], f16)
            nc.vector.memset(zrow[:], 0.0)

            cal_d = dram.tile([12, SCRROWS, SW, W2], f16)
            stat_in = dram.tile([1, 24], f32)
            stat_out = dram.tile([1, 24], f32)
            mr_rd = dram.tile([NQ, 12, 1], f32)
            mr_nd = dram.tile([NQ, 12, 1], f32)

            # ---------------- phase A: gaussian pyramid + BN partial stats
            with (
                tc.tile_pool(name="stage", bufs=2) as stage,
                tc.tile_pool(name="psumA", bufs=4, space="PSUM") as psumA,
            ):
                for w in range(NWIN):
                    st = stage.tile([WJ, 12, NF], f16, tag="st")
                    for ch in range(12):
                        src = fsw_s if ch == 11 else fyw_s
                        ps = psumA.tile([WJ, NF], f32, tag="psA")
                        nc.tensor.matmul(
                            ps[:], lhsT=toep_s[:, ch, :], rhs=src[:, w, :],
                            start=True, stop=True)
                        nc.scalar.copy(st[:, ch, :], ps[:])
                    for s in range(SW):
                        nc.sync.dma_start(
                            out=cal_d[:, 3 + WJ * w:3 + WJ * w + WJ, s, 1:1 + W]
                            .rearrange("ch r c -> r ch c"),
                            in_=st[:, :, W * s:W * s + W])
                    vr = WJ if w < NWIN - 1 else HI - WJ * (NWIN - 1)  # 54 / 20
                    red1 = stage.tile([WJ, 12], f32, tag="red1")
                    nc.vector.tensor_reduce(
                        out=red1[0:vr], in_=st[0:vr], axis=AXX, op=ADD)
                    sq = stage.tile([WJ, 12, NF], f16, tag="sq")
                    nc.scalar.activation(out=sq[0:vr], in_=st[0:vr], func=Square)
                    red2 = stage.tile([WJ, 12], f32, tag="red2")
                    nc.vector.tensor_reduce(
                        out=red2[0:vr], in_=sq[0:vr], axis=AXX, op=ADD)
                    nc.vector.tensor_tensor(
                        out=acc[0:vr, 0:12], in0=acc[0:vr, 0:12],
                        in1=red1[0:vr], op=ADD)
                    nc.vector.tensor_tensor(
                        out=acc[0:vr, 12:24], in0=acc[0:vr, 12:24],
                        in1=red2[0:vr], op=ADD)

                # ---------------- BN stats: partition-sum, allreduce, fold
                ps1 = psumA.tile([1, 24], f32, tag="ps1")
                nc.tensor.matmul(ps1[:], lhsT=ones54[:], rhs=acc[:],
                                 start=True, stop=True)
                stat_sb = singles.tile([1, 24], f32)
                nc.scalar.copy(stat_sb[:], ps1[:])
            nc.gpsimd.dma_start(stat_in[:], stat_sb[:])
            nc.gpsimd.collective_compute(
                "AllReduce", ADD, replica_groups=[list(range(NCORES))],
                ins=[stat_in.opt()], outs=[stat_out.opt()])
            gl = singles.tile([1, 24], f32)
            nc.gpsimd.dma_start(gl[:], stat_out[:])
            mt = singles.tile([1, 12], f32)
            nc.vector.tensor_scalar_mul(mt[:], gl[:, 0:12], INVN)
            var = singles.tile([1, 12], f32)
            nc.vector.tensor_tensor(out=var[:], in0=mt[:], in1=mt[:], op=MULT)
            e2 = singles.tile([1, 12], f32)
            nc.vector.tensor_scalar_mul(e2[:], gl[:, 12:24], INVN)
            nc.vector.tensor_tensor(out=var[:], in0=e2[:], in1=var[:], op=SUB)
            eps_t = singles.tile([1, 1], f32)
            nc.vector.memset(eps_t[:], BN_EPS)
            sd = singles.tile([1, 12], f32)
            nc.scalar.activation(out=sd[:], in_=var[:], func=Sqrt,
                                 bias=eps_t[:, 0:1])
            rr_t = singles.tile([1, 12], f32)
            nc.vector.reciprocal(rr_t[:], sd[:])
            nmr = singles.tile([1, 12], f32)
            nc.vector.tensor_tensor(out=nmr[:], in0=mt[:], in1=rr_t[:], op=MULT)
            nc.vector.tensor_scalar_mul(nmr[:], nmr[:], -1.0)
            for q in range(NQ):
                nc.sync.dma_start(out=mr_rd[q:q + 1, :, 0], in_=rr_t[0:1, :])
                nc.sync.dma_start(out=mr_nd[q:q + 1, :, 0], in_=nmr[0:1, :])
            r48 = singles.tile([48, 1], f32)
            nc.sync.dma_start(
                out=r48[:], in_=mr_rd[:].rearrange("q c one -> (q c) one"))
            nmr48 = singles.tile([48, 1], f32)
            nc.sync.dma_start(
                out=nmr48[:], in_=mr_nd[:].rearrange("q c one -> (q c) one"))

            # ---------------- phase B: normalize + 3x3 conv stack
            with (
                tc.tile_pool(name="io", bufs=2) as io,
                tc.tile_pool(name="acts", bufs=2) as acts,
                tc.tile_pool(name="psum", bufs=6, space="PSUM") as psum,
                tc.tile_pool(name="psum3", bufs=2, space="PSUM") as psum3,
            ):
                for st_i in range(NST):
                    s_i, t_i = divmod(st_i, NT)
                    calr = io.tile([48, CAL_SZ], f16, tag="calr")
                    for q in range(NQ):
                        r0 = R * t_i + QROWS * q
                        nc.sync.dma_start(
                            out=calr[12 * q:12 * q + 12, 1:1 + CAL_F]
                            .rearrange("p (r c) -> p r c", c=W2),
                            in_=cal_d[:, r0:r0 + CAL_ROWS, s_i, :])
                    caln = io.tile([48, CAL_SZ], f16, tag="caln")
                    nc.scalar.activation(
                        out=caln[:], in_=calr[:], func=Ident,
                        scale=r48[:, 0:1], bias=nmr48[:, 0:1])
                    cv = caln[:, 1:1 + CAL_F].rearrange("p (r c) -> p r c", c=W2)
                    nc.vector.memset(cv[:, :, 0:1], 0.0)
                    nc.vector.memset(cv[:, :, W2 - 1:W2], 0.0)
                    nc.vector.memset(caln[:, 0:1], 0.0)
                    nc.vector.memset(caln[:, 1 + CAL_F:], 0.0)
                    if t_i == 0:      # swath top: zero pad rows of quarter 0
                        nc.vector.memset(caln[0:12, 1:1 + 3 * W2], 0.0)
                    if t_i == NT - 1:  # swath bottom: zero pad rows of quarter 3
                        # (DMA: compute-engine APs need 32-aligned partition start)
                        nc.sync.dma_start(
                            out=caln[36:48, 1 + (CAL_ROWS - 3) * W2:1 + CAL_F],
                            in_=zrow[:])

                    h1 = acts.tile([128, H1_SZ], f16, tag="h1")
                    h2 = acts.tile([128, H2_SZ], f16, tag="h2")
                    ot = io.tile([4, O_F], f16, tag="ot")

                    # ---- conv1: caln[48] -> h1[128], ReLU(. + b1)
                    for off, sz in _chunks(H1_F):
                        ps = psum.tile([128, CHUNK], f32, tag="ps")
                        for t9 in range(9):
                            dy, dx = t9 // 3 - 1, t9 % 3 - 1
                            base = off + W2 * (1 + dy) + dx + 1
                            nc.tensor.matmul(
                                ps[:, :sz], lhsT=l1s[:, t9, :],
                                rhs=caln[:, base:base + sz],
                                start=(t9 == 0), stop=(t9 == 8),
                            )
                        nc.scalar.activation(
                            out=h1[:, 1 + off:1 + off + sz], in_=ps[:, :sz],
                            func=Relu, bias=b1s[:, 0:1], scale=1.0,
                        )
                    h1v = h1[:, 1:1 + H1_F].rearrange("p (r c) -> p r c", c=W2)
                    nc.vector.memset(h1v[:, :, 0:1], 0.0)
                    nc.vector.memset(h1v[:, :, W2 - 1:W2], 0.0)
                    if t_i == 0:
                        nc.vector.memset(h1[0:32, 1:1 + 2 * W2], 0.0)
                    if t_i == NT - 1:
                        nc.vector.memset(
                            h1[96:128, 1 + (H1_ROWS - 2) * W2:1 + H1_F], 0.0)

                    # ---- conv2: h1[128] -> h2[128], ReLU(. + b2)
                    for off, sz in _chunks(H2_F):
                        ps = psum.tile([128, CHUNK], f32, tag="ps")
                        for t9 in range(9):
                            dy, dx = t9 // 3 - 1, t9 % 3 - 1
                            base = off + W2 * (1 + dy) + dx + 1
                            nc.tensor.matmul(
                                ps[:, :sz], lhsT=l2s[:, t9, :],
                                rhs=h1[:, base:base + sz],
                                start=(t9 == 0), stop=(t9 == 8),
                            )
                        nc.scalar.activation(
                            out=h2[:, 1 + off:1 + off + sz], in_=ps[:, :sz],
                            func=Relu, bias=b2s[:, 0:1], scale=1.0,
                        )
                    h2v = h2[:, 1:1 + H2_F].rearrange("p (r c) -> p r c", c=W2)
                    nc.vector.memset(h2v[:, :, 0:1], 0.0)
                    nc.vector.memset(h2v[:, :, W2 - 1:W2], 0.0)
                    if t_i == 0:
                        nc.vector.memset(h2[0:32, 1:1 + W2], 0.0)
                    if t_i == NT - 1:
                        nc.vector.memset(
                            h2[96:128, 1 + (H2_ROWS - 1) * W2:1 + H2_F], 0.0)

                    # ---- conv3: h2[128] -> o[4], Identity(. + b3')
                    for off, sz in _chunks(O_F):
                        ps = psum3.tile([4, CHUNK], f32, tag="ps3")
                        for t9 in range(9):
                            dy, dx = t9 // 3 - 1, t9 % 3 - 1
                            base = off + W2 * (1 + dy) + dx + 1
                            nc.tensor.matmul(
                                ps[:, :sz], lhsT=l3s[:, t9, :],
                                rhs=h2[:, base:base + sz],
                                start=(t9 == 0), stop=(t9 == 8),
                            )
                        nc.scalar.activation(
                            out=ot[:, off:off + sz], in_=ps[:, :sz],
                            func=Ident, bias=b3s[:, 0:1], scale=1.0,
                        )
                    nc.sync.dma_start(out=o[st_i], in_=ot[:])
    _split_waits(nc)
    return nc


# ---------------------------------------------------------------- emulation
def _emulate(in_maps):
    """Numpy mirror of the fused device kernel (fp16 casts at tile edges)."""
    f16 = np.float16
    stats = np.zeros((1, 24), np.float32)
    cores = []
    for m in in_maps:
        blob = m["blob"]
        fyp = blob[OFF_FY:OFF_FS].astype(np.float32).reshape(NPAD, NF)
        fsp = blob[OFF_FS:OFF_BANDS].astype(np.float32).reshape(NPAD, NF)
        bands = blob[OFF_BANDS:OFF_W1].astype(np.float32).reshape(SIZE, 12)
        toep = np.zeros((128, 12, WJ), np.float32)
        for j in range(WJ):
            toep[j:j + SIZE, :, j] = bands
        cal_d = np.zeros((12, SCRROWS, SW, W2), np.float32)
        acc = np.zeros((WJ, 24), np.float32)
        for w in range(NWIN):
            st = np.zeros((WJ, 12, NF), np.float32)
            for ch in range(12):
                src = fsp if ch == 11 else fyp
                st[:, ch, :] = toep[:, ch, :].T @ src[WJ * w:WJ * w + 128, :]
            st = st.astype(f16).astype(np.float32)
            cal_d[:, 3 + WJ * w:3 + WJ * w + WJ, :, 1:1 + W] = (
                st.reshape(WJ, 12, SW, W).transpose(1, 0, 2, 3))
            vr = WJ if w < NWIN - 1 else HI - WJ * (NWIN - 1)
            acc[0:vr, 0:12] += st[0:vr].sum(2)
            sq = (st[0:vr] ** 2).astype(f16).astype(np.float32)
            acc[0:vr, 12:24] += sq.sum(2)
        stats += acc.sum(0, keepdims=True)
        cores.append(cal_d)
    mt = stats[:, 0:12] * INVN
    var = stats[:, 12:24] * INVN - mt * mt
    rr = 1.0 / np.sqrt(var + BN_EPS)
    nmr = -(mt * rr)
    r48 = np.tile(rr[0], NQ)[:, None]
    nmr48 = np.tile(nmr[0], NQ)[:, None]

    outs = []
    for m, cal_d in zip(in_maps, cores):
        l1 = np.zeros((9, 48, 128), np.float32)
        l2 = np.zeros((9, 128, 128), np.float32)
        l3 = np.zeros((9, 128, 4), np.float32)
        blob = m["blob"]
        w1t = blob[OFF_W1:OFF_W2].astype(np.float32).reshape(9, 12, HID)
        w2t = blob[OFF_W2:OFF_W3].astype(np.float32).reshape(9, HID, HID)
        w3t = blob[OFF_W3:OFF_B1].astype(np.float32).reshape(9, HID, 1)
        b1v = blob[OFF_B1:OFF_B2].view(np.float32)
        b2v = blob[OFF_B2:OFF_B3].view(np.float32)
        b3v = blob[OFF_B3:OFF_B3 + 2].view(np.float32)
        for t9 in range(9):
            for q in range(NQ):
                l1[t9, 12 * q:12 * q + 12, 32 * q:32 * q + 32] = w1t[t9]
                l2[t9, 32 * q:32 * q + 32, 32 * q:32 * q + 32] = w2t[t9]
                l3[t9, 32 * q:32 * q + 32, q] = w3t[t9][:, 0]
        b1s = np.tile(b1v, NQ)[:, None]
        b2s = np.tile(b2v, NQ)[:, None]
        b3s = np.full((4, 1), b3v[0], np.float32)
        o = np.zeros((NST, 4, O_F), np.float32)
        for st_i in range(NST):
            s_i, t_i = divmod(st_i, NT)
            calr = np.zeros((48, CAL_SZ), np.float32)
            for q in range(NQ):
                r0 = R * t_i + QROWS * q
                calr[12 * q:12 * q + 12, 1:1 + CAL_F] = (
                    cal_d[:, r0:r0 + CAL_ROWS, s_i, :].reshape(12, CAL_F))
            caln = (calr * r48 + nmr48).astype(f16).astype(np.float32)
            cv = caln[:, 1:1 + CAL_F].reshape(48, CAL_ROWS, W2)
            cv[:, :, 0] = 0.0
            cv[:, :, W2 - 1] = 0.0
            caln[:, 0] = 0.0
            caln[:, 1 + CAL_F:] = 0.0
            if t_i == 0:
                caln[0:12, 1:1 + 3 * W2] = 0.0
            if t_i == NT - 1:
                caln[36:48, 1 + (CAL_ROWS - 3) * W2:1 + CAL_F] = 0.0
            h1 = np.zeros((128, H1_SZ), np.float32)
            acc9 = np.zeros((128, H1_F), np.float32)
            for t9 in range(9):
                dy, dx = t9 // 3 - 1, t9 % 3 - 1
                base = W2 * (1 + dy) + dx + 1
                acc9 += l1[t9].T @ caln[:, base:base + H1_F]
            h1[:, 1:1 + H1_F] = np.maximum(acc9 + b1s, 0.0)
            h1 = h1.astype(f16).astype(np.float32)
            h1v = h1[:, 1:1 + H1_F].reshape(128, H1_ROWS, W2)
            h1v[:, :, 0] = 0.0
            h1v[:, :, W2 - 1] = 0.0
            if t_i == 0:
                h1[0:32, 1:1 + 2 * W2] = 0.0
            if t_i == NT - 1:
                h1[96:128, 1 + (H1_ROWS - 2) * W2:1 + H1_F] = 0.0
            h2 = np.zeros((128, H2_SZ), np.float32)
            acc9 = np.zeros((128, H2_F), np.float32)
            for t9 in range(9):
                dy, dx = t9 // 3 - 1, t9 % 3 - 1
                base = W2 * (1 + dy) + dx + 1
                acc9 += l2[t9].T @ h1[:, base:base + H2_F]
            h2[:, 1:1 + H2_F] = np.maximum(acc9 + b2s, 0.0)
            h2 = h2.astype(f16).astype(np.float32)
            h2v = h2[:, 1:1 + H2_F].reshape(128, H2_ROWS, W2)
            h2v[:, :, 0] = 0.0
            h2v[:, :, W2 - 1] = 0.0
            if t_i == 0:
                h2[0:32, 1:1 + W2] = 0.0
            if t_i == NT - 1:
                h2[96:128, 1 + (H2_ROWS - 1) * W2:1 + H2_F] = 0.0
            acc9 = np.zeros((4, O_F), np.float32)
            for t9 in range(9):
                dy, dx = t9 // 3 - 1, t9 % 3 - 1
                base = W2 * (1 + dy) + dx + 1
                acc9 += l3[t9].T @ h2[:, base:base + O_F]
            o[st_i] = acc9 + b3s
        outs.append({"o": o.astype(f16)})
    return outs


def _cache_path():
    import hashlib
    import inspect
    src = inspect.getsource(_build) + repr(
        (NBLOB, NST, O_F, NCORES, "v3"))
    h = hashlib.sha256(src.encode()).hexdigest()[:16]
    return f"/tmp/trn_cal_bir_{h}.pkl"


def _normalize_bir(raw):
    """Blank debug filename/lineno/traceback fields so the BIR bytes (and
    the NEFF content hash) don't depend on where kernel.py lives."""
    import re
    raw = re.sub(rb'"filename":"(?:[^"\\]|\\.)*"', b'"filename":"k"', raw)
    raw = re.sub(rb'"lineno":\d+', b'"lineno":0', raw)
    raw = re.sub(rb'"ant_traceback":"(?:[^"\\]|\\.)*"',
                 b'"ant_traceback":""', raw)
    return raw


def _get_ncobj():
    """Real Bass on cold path; lightweight shim from disk cache when warm
    (skips ISA cffi init + op building + tile scheduling, ~1s)."""
    import os
    import pickle
    from types import SimpleNamespace

    class _NcShim:
        has_collectives = True
        target_bir_lowering = False
        dbg_addr = None
        debug = False

        def __init__(self, meta):
            self._bir = meta["bir"]
            self.partition_id_tensor = SimpleNamespace(name=meta["partition"])
            self.m = SimpleNamespace(arch=meta["arch"])

        def to_json_bytes(self):
            return self._bir

        def __hash__(self):
            return hash(id(self))

        def __eq__(self, other):
            return self is other

    path = _cache_path()
    if os.path.exists(path):
        try:
            with open(path, "rb") as f:
                meta = pickle.load(f)
            return _NcShim(meta)
        except Exception:
            pass
    _apply_tile_patch()
    # build in a fresh thread: BIR debug info embeds the caller traceback,
    # so a constant stack keeps the BIR (and NEFF) bytes reproducible
    import threading
    box = {}

    def _tgt():
        box["nc"] = _build()

    th = threading.Thread(target=_tgt)
    th.start()
    th.join()
    nc = box["nc"]
    bir = _normalize_bir(nc.to_json_bytes())
    nc.to_json_bytes = lambda: bir  # lowering must embed the same bytes
    try:
        meta = dict(
            bir=bir,
            partition=nc.partition_id_tensor.name,
            arch=nc.m.arch,
        )
        tmp = path + f".tmp{os.getpid()}"
        with open(tmp, "wb") as f:
            pickle.dump(meta, f)
        os.replace(tmp, path)
    except Exception:
        pass
    return nc


def _run(in_maps):
    if EMULATE:
        return _emulate(in_maps)
    import time as _time
    import jax
    from jax.sharding import Mesh, PartitionSpec
    from jax.experimental.shard_map import shard_map
    try:
        jax.config.update("jax_compilation_cache_dir", "/tmp/jaxcache")
        jax.config.update("jax_persistent_cache_min_compile_time_secs", 0.0)
        jax.config.update("jax_persistent_cache_min_entry_size_bytes", 0)
    except Exception:
        pass
    import concourse.bass2jax as b2j

    t0 = _time.time()
    if "nc" not in _CACHE:
        _CACHE["nc"] = _get_ncobj()
    nc = _CACHE["nc"]
    b2j.install_neuronx_cc_hook()

    in_names = ["blob"]
    out_names = ["o"]
    out_avals = [jax.core.ShapedArray((NST, 4, O_F), np.float16)]
    partition_name = "partition_id"
    n_params = 1
    in_names_all = in_names + out_names + [partition_name]
    donate = (1,)

    def _body(*args):
        operands = list(args)
        operands.append(b2j.partition_id_tensor())
        outs = b2j._bass_exec_p.bind(
            *operands, out_avals=tuple(out_avals),
            in_names=tuple(in_names_all), out_names=tuple(out_names),
            lowering_input_output_aliases=(),
            sim_require_finite=True, sim_require_nnan=True, nc=nc)
        return tuple(outs)

    if "fn" not in _CACHE:
        devices = jax.devices()[:NCORES]
        assert len(devices) == NCORES
        mesh = Mesh(np.asarray(devices), ("core",))
        _CACHE["fn"] = jax.jit(
            shard_map(_body, mesh=mesh,
                      in_specs=(PartitionSpec("core"),) * 2,
                      out_specs=(PartitionSpec("core"),),
                      check_rep=False),
            donate_argnums=donate, keep_unused=True)

    concat_in = np.concatenate([m["blob"] for m in in_maps], axis=0)
    concat_zero = np.zeros((NCORES * NST, 4, O_F), np.float16)
    out_arrs = _CACHE["fn"](concat_in, concat_zero)
    o_all = np.asarray(out_arrs[0]).reshape(NCORES, NST, 4, O_F)
    _CACHE.setdefault("wall_ns", {})["fused"] = int((_time.time() - t0) * 1e9)
    return [{"o": o_all[c]} for c in range(NCORES)]


# ---------------------------------------------------------------- main entry
def kernel(sv_uncal, sv_bg, kernel, w1, b1, w2, b2, w3, b3, msk_idx, row_idx):
    sv_uncal = np.asarray(sv_uncal, np.float32)
    sv_bg = np.asarray(sv_bg, np.float32)
    w1 = np.asarray(w1, np.float32)
    b1 = np.asarray(b1, np.float32)
    w2 = np.asarray(w2, np.float32)
    b2 = np.asarray(b2, np.float32)
    w3 = np.asarray(w3, np.float32)
    b3 = np.asarray(b3, np.float32)
    msk_idx = np.asarray(msk_idx)
    row_idx = np.asarray(row_idx)

    # ---- host gather + replicate pad (zero tail to NPAD rows)
    fy = sv_uncal.reshape(B * P, H, W)[msk_idx][:, row_idx]   # [24, 1100, 52]
    fs = sv_bg.reshape(B * P, H, W)[msk_idx][:, row_idx]
    fyp = np.zeros((M_SEL, NPAD, W), np.float32)
    fsp = np.zeros((M_SEL, NPAD, W), np.float32)
    fyp[:, HALF:HALF + HI] = fy
    fsp[:, HALF:HALF + HI] = fs
    fyp[:, :HALF] = fy[:, 0:1]
    fsp[:, :HALF] = fs[:, 0:1]
    fyp[:, HALF + HI:HALF + HI + HALF] = fy[:, -1:]
    fsp[:, HALF + HI:HALF + HI + HALF] = fs[:, -1:]

    bands = _bands_from_kernel(kernel)
    bands16 = np.ascontiguousarray(bands.T[:, :, None]).astype(np.float16)
    w1f = np.concatenate(
        [w1[:, 0:10] + w1[:, 11:21], w1[:, 10:11], w1[:, 21:22]], axis=1)
    w1t = np.stack([w1f[:, :, t9 // 3, t9 % 3].T for t9 in range(9)])
    w2t = np.stack([w2[:, :, t9 // 3, t9 % 3].T for t9 in range(9)])
    w3t = np.stack([w3[0, :, t9 // 3, t9 % 3][:, None] for t9 in range(9)])
    wtail = np.empty(NBLOB - OFF_BANDS, np.float16)
    wtail[0:OFF_W1 - OFF_BANDS] = bands16.ravel()
    wtail[OFF_W1 - OFF_BANDS:OFF_W2 - OFF_BANDS] = (
        w1t.astype(np.float16).ravel())
    wtail[OFF_W2 - OFF_BANDS:OFF_W3 - OFF_BANDS] = (
        w2t.astype(np.float16).ravel())
    wtail[OFF_W3 - OFF_BANDS:OFF_B1 - OFF_BANDS] = (
        w3t.astype(np.float16).ravel())
    wtail[OFF_B1 - OFF_BANDS:OFF_B2 - OFF_BANDS] = (
        b1.astype(np.float32).view(np.float16))
    wtail[OFF_B2 - OFF_BANDS:OFF_B3 - OFF_BANDS] = (
        b2.astype(np.float32).view(np.float16))
    wtail[OFF_B3 - OFF_BANDS:] = np.asarray(
        [b3[0] + np.float32(NS[0] / NS[1])], np.float32).view(np.float16)
    in_maps = []
    for c in range(NCORES):
        sl = slice(SW * c, SW * c + SW)
        blob = np.empty(NBLOB, np.float16)
        blob[OFF_FY:OFF_FS] = np.ascontiguousarray(
            fyp[sl].transpose(1, 0, 2)).astype(np.float16).ravel()
        blob[OFF_FS:OFF_BANDS] = np.ascontiguousarray(
            fsp[sl].transpose(1, 0, 2)).astype(np.float16).ravel()
        blob[OFF_BANDS:] = wtail
        in_maps.append(dict(blob=blob))

    res = _run(in_maps)

    # ---- assemble + fs + scatter (host)
    outs = []
    for c in range(NCORES):
        oo = res[c]["o"].astype(np.float32)
        oo = oo.reshape(SW, NT, 4, R, W2)[:, :, :, :, 1:1 + W]
        outs.append(oo.transpose(0, 2, 1, 3, 4).reshape(SW, HI, W))
    o_dev = np.concatenate(outs, axis=0)                      # [24, 1100, 52]
    out = o_dev + fs

    out_cal = np.zeros((B * P, HI, W), np.float32)
    np.add.at(out_cal, msk_idx, out)
    cnt = np.zeros((B * P,), np.float32)
    np.add.at(cnt, msk_idx, 1.0)
    out_msk = np.broadcast_to(
        (cnt > 0)[:, None, None], (B * P, HI, W)).copy()
    return (out_cal.reshape(B, P, HI, W),
            out_msk.reshape(B, P, HI, W))



# revision 19
# speedup vs baseline: 91.4025x; 1.2789x over previous
"""Trainium2 Bass kernel for nn_CalibrationModelObsGridGeometry.

Single fused SPMD launch on 8 cores (3 swaths/core), one packed fp16 input
blob per core (~0.78 MB) and one fp16 output (~0.36 MB) to minimize
host<->device traffic and per-array transfer overhead:

  host:   gather + replicate-pad fy/fs, cast fp16, pack blob
  device: window via DMA -> gaussian-pyramid Toeplitz matmuls (fp16) ->
          per-core BN partial sums -> 8-core AllReduce (96 B) ->
          per-partition scale/bias normalization activation ->
          3x3 conv stack as accumulating matmuls (fp16, block-diag over
          4 row-quarters) -> fp16 output
  host:   + fs_sel + scatter-add, mask.

Toeplitz bands, block-diagonal conv weights and biases are assembled
on-device from the blob instead of shipping expanded forms.

Launch overhead is held down by (a) a /tmp disk cache of the built BIR so
warm processes skip Bass construction and tile scheduling (~1 s), (b) the
jax persistent compilation cache, and (c) debug-info normalization that
makes the BIR bytes reproducible regardless of build directory, so every
rebuild yields the identical NEFF and hits the runtime content cache.
"""

import numpy as np

# ---------------------------------------------------------------- constants
B, P, H, W = 4, 8, 1200, 52
M_SEL, HI = 24, 1100
SIZE = 75
HALF = SIZE // 2  # 37
NS = (0.31446309894037083, 0.3886609494201447)
BN_EPS = 1e-5
HID = 32
NCORES = 8
SW = 3                      # swaths per core
NWIN = 21                   # toeplitz windows per swath (54 out rows each)
WJ = 54                     # out rows per window
NPAD = WJ * (NWIN - 1) + 128  # 1208 padded input rows
NQ = 4                      # h-quarters (partition groups)
QROWS = HI // NQ            # 275
NT = 5                      # processing tiles per swath
R = QROWS // NT             # 55 out rows per tile per quarter
W2 = 54                     # padded width
CAL_ROWS = R + 6            # 61 stored cal rows per tile
H1_ROWS = R + 4             # 59
H2_ROWS = R + 2             # 57
CAL_F = CAL_ROWS * W2       # 3294
H1_F = H1_ROWS * W2         # 3186
H2_F = H2_ROWS * W2         # 3078
O_F = R * W2                # 2970
CAL_SZ = CAL_F + 2          # +1 lead, +1 tail guard
H1_SZ = H1_F + 2
H2_SZ = H2_F + 2
CHUNK = 486                 # <=512 fp32 psum-bank limit
NST = SW * NT               # 15 processing tiles per core
NF = SW * W                 # 156
INVN = 1.0 / float(M_SEL * HI * W)
SCRROWS = 3 + WJ * NWIN + 3  # 1140 cal scratch rows (3 lead, tail garbage)

EMULATE = False             # numpy-emulate the device kernel (debug)

# single packed fp16 input blob (per core): fy, fs, bands, w1t, w2t, w3t,
# b1/b2/b3 (f32 bit-cast to f16 pairs)
OFF_FY = 0
OFF_FS = OFF_FY + NPAD * NF
OFF_BANDS = OFF_FS + NPAD * NF
OFF_W1 = OFF_BANDS + SIZE * 12
OFF_W2 = OFF_W1 + 9 * 12 * HID
OFF_W3 = OFF_W2 + 9 * HID * HID
OFF_B1 = OFF_W3 + 9 * HID
OFF_B2 = OFF_B1 + 2 * HID
OFF_B3 = OFF_B2 + 2 * HID
NBLOB = OFF_B3 + 2


def _bands_from_kernel(kern):
    """12 cal channels as 75-tap bands: D0..D9, A(=G9 on fy), B(=G9 on fs)."""
    g = np.asarray(kern, np.float32).reshape(10, SIZE)
    bands = np.zeros((12, SIZE), np.float32)
    bands[0] = -g[0]
    bands[0, HALF] += 1.0
    for i in range(1, 10):
        bands[i] = g[i - 1] - g[i]
    bands[10] = g[9]
    bands[11] = g[9]
    return bands


def _chunks(total):
    out = []
    off = 0
    while off < total:
        sz = min(CHUNK, total - off)
        out.append((off, sz))
        off += sz
    return out


# ---------------------------------------------------------------- device build
_CACHE = {}


def _apply_tile_patch():
    import concourse.tile as tile
    from concourse import mybir
    from concourse.vector_clock import ScopedClock

    def _patched(self, tick_clock, wait_clock):
        nc = self.nc
        drain_inst = nc.sync.drain()
        wait_clock.add_sem_waits(
            drain_inst.ins, ScopedClock({None: tick_clock.global_clock})
        )
        si = drain_inst.ins.sync_info
        if si is not None and si.on_wait and len(si.on_wait) > 1:
            extra = list(si.on_wait[1:])
            del si.on_wait[1:]
            for w in extra:
                d2 = nc.sync.drain()
                si2 = d2.ins.sync_info
                if si2 is None:
                    d2.ins.sync_info = mybir.SyncInfo(on_wait=[w], on_update=[])
                else:
                    si2.on_wait.append(w)
        nc.all_engine_barrier()
        popped = nc._tile_sem_poison_stack.pop()
        assert popped is self._sem_poison
        nc.clear_and_free_semaphores(list(self.sems.allocated().values()))
        nc.all_engine_barrier()

    tile.TileContext._drain_and_barrier = _patched


_WSPLIT_N = [0]


def _split_waits(nc):
    """This walrus build accepts only one sync-wait per instruction: hoist
    extra waits onto same-engine NoOps placed just before the instruction."""
    from concourse import mybir
    for f in nc.m.functions:
        for bb in f.blocks:
            new_list = []
            for ins in bb.instructions:
                si = getattr(ins, "sync_info", None)
                if si is not None and si.on_wait and len(si.on_wait) > 1:
                    extra = list(si.on_wait[:-1])
                    del si.on_wait[:-1]
                    for w in extra:
                        _WSPLIT_N[0] += 1
                        nop = mybir.InstDrain(
                            name=f"WSPLIT-{_WSPLIT_N[0]}",
                            engine=ins.engine,
                            sync_info=mybir.SyncInfo(on_wait=[w], on_update=[]),
                            bass_is_fusable=False,
                        )
                        new_list.append(nop)
                new_list.append(ins)
            bb.instructions[:] = new_list


def _build():
    import concourse.bass as bass
    import concourse.tile as tile
    from concourse import mybir

    f32 = mybir.dt.float32
    f16 = mybir.dt.float16
    nc = bass.Bass("TRN2", num_devices=NCORES,
                   disable_frame_to_traceback=True)
    blob = nc.dram_tensor("blob", [NBLOB], f16, kind="ExternalInput")
    o = nc.dram_tensor("o", [NST, 4, O_F], f16, kind="ExternalOutput")

    Relu = mybir.ActivationFunctionType.Relu
    Ident = mybir.ActivationFunctionType.Identity
    Square = mybir.ActivationFunctionType.Square
    Sqrt = mybir.ActivationFunctionType.Sqrt
    ADD = mybir.AluOpType.add
    MULT = mybir.AluOpType.mult
    SUB = mybir.AluOpType.subtract
    AXX = mybir.AxisListType.X

    with tile.TileContext(nc) as tc:
        with (
            tc.tile_pool(name="singles", bufs=1) as singles,
            tc.tile_pool(name="dram", bufs=1, space="DRAM") as dram,
        ):
            # ---------------- setup: windows, toeplitz, weights
            fyw_s = singles.tile([128, NWIN, NF], f16)
            fsw_s = singles.tile([128, NWIN, NF], f16)
            for w in range(NWIN):
                nc.sync.dma_start(
                    out=fyw_s[:, w, :],
                    in_=blob[OFF_FY + NF * WJ * w:OFF_FY + NF * (WJ * w + 128)]
                    .rearrange("(r c) -> r c", c=NF))
                nc.sync.dma_start(
                    out=fsw_s[:, w, :],
                    in_=blob[OFF_FS + NF * WJ * w:OFF_FS + NF * (WJ * w + 128)]
                    .rearrange("(r c) -> r c", c=NF))
            toep_s = singles.tile([128, 12, WJ], f16)
            nc.vector.memset(toep_s[:], 0.0)
            bands_ap = blob[OFF_BANDS:OFF_BANDS + SIZE * 12].rearrange(
                "(d ch one) -> d ch one", ch=12, one=1)
            for j in range(WJ):
                nc.sync.dma_start(out=toep_s[j:j + SIZE, :, j:j + 1], in_=bands_ap)
            l1s = singles.tile([48, 9, 128], f16)
            l2s = singles.tile([128, 9, 128], f16)
            l3s = singles.tile([128, 9, 4], f16)
            nc.vector.memset(l1s[:], 0.0)
            nc.vector.memset(l2s[:], 0.0)
            nc.vector.memset(l3s[:], 0.0)
            for t9 in range(9):
                w1_ap = blob[OFF_W1 + 12 * HID * t9:OFF_W1 + 12 * HID * (t9 + 1)
                             ].rearrange("(a b) -> a b", b=HID)
                w2_ap = blob[OFF_W2 + HID * HID * t9:OFF_W2 + HID * HID * (t9 + 1)
                             ].rearrange("(a b) -> a b", b=HID)
                w3_ap = blob[OFF_W3 + HID * t9:OFF_W3 + HID * (t9 + 1)
                             ].rearrange("(a b) -> a b", b=1)
                for q in range(NQ):
                    nc.sync.dma_start(
                        out=l1s[12 * q:12 * q + 12, t9, 32 * q:32 * q + 32],
                        in_=w1_ap)
                    nc.sync.dma_start(
                        out=l2s[32 * q:32 * q + 32, t9, 32 * q:32 * q + 32],
                        in_=w2_ap)
                    nc.sync.dma_start(
                        out=l3s[32 * q:32 * q + 32, t9, q:q + 1],
                        in_=w3_ap)
            b1s = singles.tile([128, 1], f32)
            b2s = singles.tile([128, 1], f32)
            b3s = singles.tile([4, 1], f32)
            b1_ap = blob[OFF_B1:OFF_B1 + 2 * HID].rearrange(
                "(a b) -> a b", b=2).bitcast(f32)
            b2_ap = blob[OFF_B2:OFF_B2 + 2 * HID].rearrange(
                "(a b) -> a b", b=2).bitcast(f32)
            b3_ap = blob[OFF_B3:OFF_B3 + 2].rearrange(
                "(a b) -> a b", b=2).bitcast(f32)
            for q in range(NQ):
                nc.sync.dma_start(out=b1s[32 * q:32 * q + 32, :], in_=b1_ap)
                nc.sync.dma_start(out=b2s[32 * q:32 * q + 32, :], in_=b2_ap)
                nc.sync.dma_start(out=b3s[q:q + 1, :], in_=b3_ap)
            ones54 = singles.tile([WJ, 1], f32)
            nc.vector.memset(ones54[:], 1.0)
            acc = singles.tile([WJ, 24], f32)
            nc.vector.memset(acc[:], 0.0)
            zrow = singles.tile([12, 3 * W2], f16)
            nc.vector.memset(zrow[:], 0.0)

            cal_d = dram.tile([12, SCRROWS, SW, W2], f16)
            stat_in = dram.tile([1, 24], f32)
            stat_out = dram.tile([1, 24], f32)
            mr_rd = dram.tile([NQ, 12, 1], f32)
            mr_nd = dram.tile([NQ, 12, 1], f32)

            # ---------------- phase A: gaussian pyramid + BN partial stats
            with (
                tc.tile_pool(name="stage", bufs=2) as stage,
                tc.tile_pool(name="psumA", bufs=4, space="PSUM") as psumA,
            ):
                for w in range(NWIN):
                    st = stage.tile([WJ, 12, NF], f16, tag="st")
                    for ch in range(12):
                        src = fsw_s if ch == 11 else fyw_s
                        ps = psumA.tile([WJ, NF], f32, tag="psA")
                        nc.tensor.matmul(
                            ps[:], lhsT=toep_s[:, ch, :], rhs=src[:, w, :],
                            start=True, stop=True)
                        nc.scalar.copy(st[:, ch, :], ps[:])
                    for s in range(SW):
                        nc.sync.dma_start(
                            out=cal_d[:, 3 + WJ * w:3 + WJ * w + WJ, s, 1:1 + W]
                            .rearrange("ch r c -> r ch c"),
                            in_=st[:, :, W * s:W * s + W])
                    vr = WJ if w < NWIN - 1 else HI - WJ * (NWIN - 1)  # 54 / 20
                    red1 = stage.tile([WJ, 12], f32, tag="red1")
                    nc.vector.tensor_reduce(
                        out=red1[0:vr], in_=st[0:vr], axis=AXX, op=ADD)
                    sq = stage.tile([WJ, 12, NF], f16, tag="sq")
                    nc.scalar.activation(out=sq[0:vr], in_=st[0:vr], func=Square)
                    red2 = stage.tile([WJ, 12], f32, tag="red2")
                    nc.vector.tensor_reduce(
                        out=red2[0:vr], in_=sq[0:vr], axis=AXX, op=ADD)
                    nc.vector.tensor_tensor(
                        out=acc[0:vr, 0:12], in0=acc[0:vr, 0:12],
                        in1=red1[0:vr], op=ADD)
                    nc.vector.tensor_tensor(
                        out=acc[0:vr, 12:24], in0=acc[0:vr, 12:24],
                        in1=red2[0:vr], op=ADD)

                # ---------------- BN stats: partition-sum, allreduce, fold
                ps1 = psumA.tile([1, 24], f32, tag="ps1")
                nc.tensor.matmul(ps1[:], lhsT=ones54[:], rhs=acc[:],
                                 start=True, stop=True)
                stat_sb = singles.tile([1, 24], f32)
                nc.scalar.copy(stat_sb[:], ps1[:])
            nc.gpsimd.dma_start(stat_in[:], stat_sb[:])
            nc.gpsimd.collective_compute(
                "AllReduce", ADD, replica_groups=[list(range(NCORES))],
                ins=[stat_in.opt()], outs=[stat_out.opt()])
            gl = singles.tile([1, 24], f32)
            nc.gpsimd.dma_start(gl[:], stat_out[:])
            mt = singles.tile([1, 12], f32)
            nc.vector.tensor_scalar_mul(mt[:], gl[:, 0:12], INVN)
            var = singles.tile([1, 12], f32)
            nc.vector.tensor_tensor(out=var[:], in0=mt[:], in1=mt[:], op=MULT)
            e2 = singles.tile([1, 12], f32)
            nc.vector.tensor_scalar_mul(e2[:], gl[:, 12:24], INVN)
            nc.vector.tensor_tensor(out=var[:], in0=e2[:], in1=var[:], op=SUB)
            eps_t = singles.tile([1, 1], f32)
            nc.vector.memset(eps_t[:], BN_EPS)
            sd = singles.tile([1, 12], f32)
            nc.scalar.activation(out=sd[:], in_=var[:], func=Sqrt,
                                 bias=eps_t[:, 0:1])
            rr_t = singles.tile([1, 12], f32)
            nc.vector.reciprocal(rr_t[:], sd[:])
            nmr = singles.tile([1, 12], f32)
            nc.vector.tensor_tensor(out=nmr[:], in0=mt[:], in1=rr_t[:], op=MULT)
            nc.vector.tensor_scalar_mul(nmr[:], nmr[:], -1.0)
            for q in range(NQ):
                nc.sync.dma_start(out=mr_rd[q:q + 1, :, 0], in_=rr_t[0:1, :])
                nc.sync.dma_start(out=mr_nd[q:q + 1, :, 0], in_=nmr[0:1, :])
            r48 = singles.tile([48, 1], f32)
            nc.sync.dma_start(
                out=r48[:], in_=mr_rd[:].rearrange("q c one -> (q c) one"))
            nmr48 = singles.tile([48, 1], f32)
            nc.sync.dma_start(
                out=nmr48[:], in_=mr_nd[:].rearrange("q c one -> (q c) one"))

            # ---------------- phase B: normalize + 3x3 conv stack
            with (
                tc.tile_pool(name="io", bufs=2) as io,
                tc.tile_pool(name="acts", bufs=2) as acts,
                tc.tile_pool(name="psum", bufs=6, space="PSUM") as psum,
                tc.tile_pool(name="psum3", bufs=2, space="PSUM") as psum3,
            ):
                for st_i in range(NST):
                    s_i, t_i = divmod(st_i, NT)
                    calr = io.tile([48, CAL_SZ], f16, tag="calr")
                    for q in range(NQ):
                        r0 = R * t_i + QROWS * q
                        nc.sync.dma_start(
                            out=calr[12 * q:12 * q + 12, 1:1 + CAL_F]
                            .rearrange("p (r c) -> p r c", c=W2),
                            in_=cal_d[:, r0:r0 + CAL_ROWS, s_i, :])
                    caln = io.tile([48, CAL_SZ], f16, tag="caln")
                    nc.scalar.activation(
                        out=caln[:], in_=calr[:], func=Ident,
                        scale=r48[:, 0:1], bias=nmr48[:, 0:1])
                    cv = caln[:, 1:1 + CAL_F].rearrange("p (r c) -> p r c", c=W2)
                    nc.vector.memset(cv[:, :, 0:1], 0.0)
                    nc.vector.memset(cv[:, :, W2 - 1:W2], 0.0)
                    nc.vector.memset(caln[:, 0:1], 0.0)
                    nc.vector.memset(caln[:, 1 + CAL_F:], 0.0)
                    if t_i == 0:      # swath top: zero pad rows of quarter 0
                        nc.vector.memset(caln[0:12, 1:1 + 3 * W2], 0.0)
                    if t_i == NT - 1:  # swath bottom: zero pad rows of quarter 3
                        # (DMA: compute-engine APs need 32-aligned partition start)
                        nc.sync.dma_start(
                            out=caln[36:48, 1 + (CAL_ROWS - 3) * W2:1 + CAL_F],
                            in_=zrow[:])

                    h1 = acts.tile([128, H1_SZ], f16, tag="h1")
                    h2 = acts.tile([128, H2_SZ], f16, tag="h2")
                    ot = io.tile([4, O_F], f16, tag="ot")

                    # ---- conv1: caln[48] -> h1[128], ReLU(. + b1)
                    for off, sz in _chunks(H1_F):
                        ps = psum.tile([128, CHUNK], f32, tag="ps")
                        for t9 in range(9):
                            dy, dx = t9 // 3 - 1, t9 % 3 - 1
                            base = off + W2 * (1 + dy) + dx + 1
                            nc.tensor.matmul(
                                ps[:, :sz], lhsT=l1s[:, t9, :],
                                rhs=caln[:, base:base + sz],
                                start=(t9 == 0), stop=(t9 == 8),
                            )
                        nc.scalar.activation(
                            out=h1[:, 1 + off:1 + off + sz], in_=ps[:, :sz],
                            func=Relu, bias=b1s[:, 0:1], scale=1.0,
                        )
                    h1v = h1[:, 1:1 + H1_F].rearrange("p (r c) -> p r c", c=W2)
                    nc.vector.memset(h1v[:, :, 0:1], 0.0)
                    nc.vector.memset(h1v[:, :, W2 - 1:W2], 0.0)
                    if t_i == 0:
                        nc.vector.memset(h1[0:32, 1:1 + 2 * W2], 0.0)
                    if t_i == NT - 1:
                        nc.vector.memset(
                            h1[96:128, 1 + (H1_ROWS - 2) * W2:1 + H1_F], 0.0)

                    # ---- conv2: h1[128] -> h2[128], ReLU(. + b2)
                    for off, sz in _chunks(H2_F):
                        ps = psum.tile([128, CHUNK], f32, tag="ps")
                        for t9 in range(9):
                            dy, dx = t9 // 3 - 1, t9 % 3 - 1
                            base = off + W2 * (1 + dy) + dx + 1
                            nc.tensor.matmul(
                                ps[:, :sz], lhsT=l2s[:, t9, :],
                                rhs=h1[:, base:base + sz],
                                start=(t9 == 0), stop=(t9 == 8),
                            )
                        nc.scalar.activation(
                            out=h2[:, 1 + off:1 + off + sz], in_=ps[:, :sz],
                            func=Relu, bias=b2s[:, 0:1], scale=1.0,
                        )
                    h2v = h2[:, 1:1 + H2_F].rearrange("p (r c) -> p r c", c=W2)
                    nc.vector.memset(h2v[:, :, 0:1], 0.0)
                    nc.vector.memset(h2v[:, :, W2 - 1:W2], 0.0)
                    if t_i == 0:
                        nc.vector.memset(h2[0:32, 1:1 + W2], 0.0)
                    if t_i == NT - 1:
                        nc.vector.memset(
                            h2[96:128, 1 + (H2_ROWS - 1) * W2:1 + H2_F], 0.0)

                    # ---- conv3: h2[128] -> o[4], Identity(. + b3')
                    for off, sz in _chunks(O_F):
                        ps = psum3.tile([4, CHUNK], f32, tag="ps3")
                        for t9 in range(9):
                            dy, dx = t9 // 3 - 1, t9 % 3 - 1
                            base = off + W2 * (1 + dy) + dx + 1
                            nc.tensor.matmul(
                                ps[:, :sz], lhsT=l3s[:, t9, :],
                                rhs=h2[:, base:base + sz],
                                start=(t9 == 0), stop=(t9 == 8),
                            )
                        nc.scalar.activation(
                            out=ot[:, off:off + sz], in_=ps[:, :sz],
                            func=Ident, bias=b3s[:, 0:1], scale=1.0,
                        )
                    nc.sync.dma_start(out=o[st_i], in_=ot[:])
    _split_waits(nc)
    return nc


# ---------------------------------------------------------------- emulation
def _emulate(in_maps):
    """Numpy mirror of the fused device kernel (fp16 casts at tile edges)."""
    f16 = np.float16
    stats = np.zeros((1, 24), np.float32)
    cores = []
    for m in in_maps:
        blob = m["blob"]
        fyp = blob[OFF_FY:OFF_FS].astype(np.float32).reshape(NPAD, NF)
        fsp = blob[OFF_FS:OFF_BANDS].astype(np.float32).reshape(NPAD, NF)
        bands = blob[OFF_BANDS:OFF_W1].astype(np.float32).reshape(SIZE, 12)
        toep = np.zeros((128, 12, WJ), np.float32)
        for j in range(WJ):
            toep[j:j + SIZE, :, j] = bands
        cal_d = np.zeros((12, SCRROWS, SW, W2), np.float32)
        acc = np.zeros((WJ, 24), np.float32)
        for w in range(NWIN):
            st = np.zeros((WJ, 12, NF), np.float32)
            for ch in range(12):
                src = fsp if ch == 11 else fyp
                st[:, ch, :] = toep[:, ch, :].T @ src[WJ * w:WJ * w + 128, :]
            st = st.astype(f16).astype(np.float32)
            cal_d[:, 3 + WJ * w:3 + WJ * w + WJ, :, 1:1 + W] = (
                st.reshape(WJ, 12, SW, W).transpose(1, 0, 2, 3))
            vr = WJ if w < NWIN - 1 else HI - WJ * (NWIN - 1)
            acc[0:vr, 0:12] += st[0:vr].sum(2)
            sq = (st[0:vr] ** 2).astype(f16).astype(np.float32)
            acc[0:vr, 12:24] += sq.sum(2)
        stats += acc.sum(0, keepdims=True)
        cores.append(cal_d)
    mt = stats[:, 0:12] * INVN
    var = stats[:, 12:24] * INVN - mt * mt
    rr = 1.0 / np.sqrt(var + BN_EPS)
    nmr = -(mt * rr)
    r48 = np.tile(rr[0], NQ)[:, None]
    nmr48 = np.tile(nmr[0], NQ)[:, None]

    outs = []
    for m, cal_d in zip(in_maps, cores):
        l1 = np.zeros((9, 48, 128), np.float32)
        l2 = np.zeros((9, 128, 128), np.float32)
        l3 = np.zeros((9, 128, 4), np.float32)
        blob = m["blob"]
        w1t = blob[OFF_W1:OFF_W2].astype(np.float32).reshape(9, 12, HID)
        w2t = blob[OFF_W2:OFF_W3].astype(np.float32).reshape(9, HID, HID)
        w3t = blob[OFF_W3:OFF_B1].astype(np.float32).reshape(9, HID, 1)
        b1v = blob[OFF_B1:OFF_B2].view(np.float32)
        b2v = blob[OFF_B2:OFF_B3].view(np.float32)
        b3v = blob[OFF_B3:OFF_B3 + 2].view(np.float32)
        for t9 in range(9):
            for q in range(NQ):
                l1[t9, 12 * q:12 * q + 12, 32 * q:32 * q + 32] = w1t[t9]
                l2[t9, 32 * q:32 * q + 32, 32 * q:32 * q + 32] = w2t[t9]
                l3[t9, 32 * q:32 * q + 32, q] = w3t[t9][:, 0]
        b1s = np.tile(b1v, NQ)[:, None]
        b2s = np.tile(b2v, NQ)[:, None]
        b3s = np.full((4, 1), b3v[0], np.float32)
        o = np.zeros((NST, 4, O_F), np.float32)
        for st_i in range(NST):
            s_i, t_i = divmod(st_i, NT)
            calr = np.zeros((48, CAL_SZ), np.float32)
            for q in range(NQ):
                r0 = R * t_i + QROWS * q
                calr[12 * q:12 * q + 12, 1:1 + CAL_F] = (
                    cal_d[:, r0:r0 + CAL_ROWS, s_i, :].reshape(12, CAL_F))
            caln = (calr * r48 + nmr48).astype(f16).astype(np.float32)
            cv = caln[:, 1:1 + CAL_F].reshape(48, CAL_ROWS, W2)
            cv[:, :, 0] = 0.0
            cv[:, :, W2 - 1] = 0.0
            caln[:, 0] = 0.0
            caln[:, 1 + CAL_F:] = 0.0
            if t_i == 0:
                caln[0:12, 1:1 + 3 * W2] = 0.0
            if t_i == NT - 1:
                caln[36:48, 1 + (CAL_ROWS - 3) * W2:1 + CAL_F] = 0.0
            h1 = np.zeros((128, H1_SZ), np.float32)
            acc9 = np.zeros((128, H1_F), np.float32)
            for t9 in range(9):
                dy, dx = t9 // 3 - 1, t9 % 3 - 1
                base = W2 * (1 + dy) + dx + 1
                acc9 += l1[t9].T @ caln[:, base:base + H1_F]
            h1[:, 1:1 + H1_F] = np.maximum(acc9 + b1s, 0.0)
            h1 = h1.astype(f16).astype(np.float32)
            h1v = h1[:, 1:1 + H1_F].reshape(128, H1_ROWS, W2)
            h1v[:, :, 0] = 0.0
            h1v[:, :, W2 - 1] = 0.0
            if t_i == 0:
                h1[0:32, 1:1 + 2 * W2] = 0.0
            if t_i == NT - 1:
                h1[96:128, 1 + (H1_ROWS - 2) * W2:1 + H1_F] = 0.0
            h2 = np.zeros((128, H2_SZ), np.float32)
            acc9 = np.zeros((128, H2_F), np.float32)
            for t9 in range(9):
                dy, dx = t9 // 3 - 1, t9 % 3 - 1
                base = W2 * (1 + dy) + dx + 1
                acc9 += l2[t9].T @ h1[:, base:base + H2_F]
            h2[:, 1:1 + H2_F] = np.maximum(acc9 + b2s, 0.0)
            h2 = h2.astype(f16).astype(np.float32)
            h2v = h2[:, 1:1 + H2_F].reshape(128, H2_ROWS, W2)
            h2v[:, :, 0] = 0.0
            h2v[:, :, W2 - 1] = 0.0
            if t_i == 0:
                h2[0:32, 1:1 + W2] = 0.0
            if t_i == NT - 1:
                h2[96:128, 1 + (H2_ROWS - 1) * W2:1 + H2_F] = 0.0
            acc9 = np.zeros((4, O_F), np.float32)
            for t9 in range(9):
                dy, dx = t9 // 3 - 1, t9 % 3 - 1
                base = W2 * (1 + dy) + dx + 1
                acc9 += l3[t9].T @ h2[:, base:base + O_F]
            o[st_i] = acc9 + b3s
        outs.append({"o": o.astype(f16)})
    return outs


def _cache_path():
    import hashlib
    import inspect
    src = inspect.getsource(_build) + repr(
        (NBLOB, NST, O_F, NCORES, "v3"))
    h = hashlib.sha256(src.encode()).hexdigest()[:16]
    return f"/tmp/trn_cal_bir_{h}.pkl"


def _normalize_bir(raw):
    """Blank debug filename/lineno/traceback fields so the BIR bytes (and
    the NEFF content hash) don't depend on where kernel.py lives."""
    import re
    raw = re.sub(rb'"filename":"(?:[^"\\]|\\.)*"', b'"filename":"k"', raw)
    raw = re.sub(rb'"lineno":\d+', b'"lineno":0', raw)
    raw = re.sub(rb'"ant_traceback":"(?:[^"\\]|\\.)*"',
                 b'"ant_traceback":""', raw)
    return raw


def _get_ncobj():
    """Real Bass on cold path; lightweight shim from disk cache when warm
    (skips ISA cffi init + op building + tile scheduling, ~1s)."""
    import os
    import pickle
    from types import SimpleNamespace

    class _NcShim:
        has_collectives = True
        target_bir_lowering = False
        dbg_addr = None
        debug = False

        def __init__(self, meta):
            self._bir = meta["bir"]
            self.partition_id_tensor = SimpleNamespace(name=meta["partition"])
            self.m = SimpleNamespace(arch=meta["arch"])

        def to_json_bytes(self):
            return self._bir

        def __hash__(self):
            return hash(id(self))

        def __eq__(self, other):
            return self is other

    path = _cache_path()
    if os.path.exists(path):
        try:
            with open(path, "rb") as f:
                meta = pickle.load(f)
            return _NcShim(meta)
        except Exception:
            pass
    _apply_tile_patch()
    # build in a fresh thread: BIR debug info embeds the caller traceback,
    # so a constant stack keeps the BIR (and NEFF) bytes reproducible
    import threading
    box = {}

    def _tgt():
        box["nc"] = _build()

    th = threading.Thread(target=_tgt)
    th.start()
    th.join()
    nc = box["nc"]
    bir = _normalize_bir(nc.to_json_bytes())
    nc.to_json_bytes = lambda: bir  # lowering must embed the same bytes
    try:
        meta = dict(
            bir=bir,
            partition=nc.partition_id_tensor.name,
            arch=nc.m.arch,
        )
        tmp = path + f".tmp{os.getpid()}"
        with open(tmp, "wb") as f:
            pickle.dump(meta, f)
        os.replace(tmp, path)
    except Exception:
        pass
    return nc


def _run(in_maps):
    if EMULATE:
        return _emulate(in_maps)
    import time as _time
    import jax
    from jax.sharding import Mesh, PartitionSpec
    from jax.experimental.shard_map import shard_map
    try:
        jax.config.update("jax_compilation_cache_dir", "/tmp/jaxcache")
        jax.config.update("jax_persistent_cache_min_compile_time_secs", 0.0)
        jax.config.update("jax_persistent_cache_min_entry_size_bytes", 0)
    except Exception:
        pass
    import concourse.bass2jax as b2j

    t0 = _time.time()
    if "nc" not in _CACHE:
        _CACHE["nc"] = _get_ncobj()
    nc = _CACHE["nc"]
    b2j.install_neuronx_cc_hook()

    in_names = ["blob"]
    out_names = ["o"]
    out_avals = [jax.core.ShapedArray((NST, 4, O_F), np.float16)]
    partition_name = "partition_id"
    n_params = 1
    in_names_all = in_names + out_names + [partition_name]
    donate = (1,)

    def _body(*args):
        operands = list(args)
        operands.append(b2j.partition_id_tensor())
        outs = b2j._bass_exec_p.bind(
            *operands, out_avals=tuple(out_avals),
            in_names=tuple(in_names_all), out_names=tuple(out_names),
            lowering_input_output_aliases=(),
            sim_require_finite=True, sim_require_nnan=True, nc=nc)
        return tuple(outs)

    if "fn" not in _CACHE:
        devices = jax.devices()[:NCORES]
        assert len(devices) == NCORES
        mesh = Mesh(np.asarray(devices), ("core",))
        _CACHE["fn"] = jax.jit(
            shard_map(_body, mesh=mesh,
                      in_specs=(PartitionSpec("core"),) * 2,
                      out_specs=(PartitionSpec("core"),),
                      check_rep=False),
            donate_argnums=donate, keep_unused=True)

    concat_in = np.concatenate([m["blob"] for m in in_maps], axis=0)
    concat_zero = np.zeros((NCORES * NST, 4, O_F), np.float16)
    out_arrs = _CACHE["fn"](concat_in, concat_zero)
    o_all = np.asarray(out_arrs[0]).reshape(NCORES, NST, 4, O_F)
    _CACHE.setdefault("wall_ns", {})["fused"] = int((_time.time() - t0) * 1e9)
    return [{"o": o_all[c]} for c in range(NCORES)]


# ---------------------------------------------------------------- main entry
def kernel(sv_uncal, sv_bg, kernel, w1, b1, w2, b2, w3, b3, msk_idx, row_idx):
    sv_uncal = np.asarray(sv_uncal, np.float32)
    sv_bg = np.asarray(sv_bg, np.float32)
    w1 = np.asarray(w1, np.float32)
    b1 = np.asarray(b1, np.float32)
    w2 = np.asarray(w2, np.float32)
    b2 = np.asarray(b2, np.float32)
    w3 = np.asarray(w3, np.float32)
    b3 = np.asarray(b3, np.float32)
    msk_idx = np.asarray(msk_idx)
    row_idx = np.asarray(row_idx)

    if not EMULATE and "warm" not in _CACHE:
        _CACHE["warm"] = True
        import threading

        def _warm():
            try:
                import jax
                jax.devices()
            except Exception:
                pass
        threading.Thread(target=_warm, daemon=True).start()

    # ---- host gather + replicate pad (zero tail to NPAD rows)
    fy = sv_uncal.reshape(B * P, H, W)[msk_idx][:, row_idx]   # [24, 1100, 52]
    fs = sv_bg.reshape(B * P, H, W)[msk_idx][:, row_idx]
    fyp = np.zeros((M_SEL, NPAD, W), np.float32)
    fsp = np.zeros((M_SEL, NPAD, W), np.float32)
    fyp[:, HALF:HALF + HI] = fy
    fsp[:, HALF:HALF + HI] = fs
    fyp[:, :HALF] = fy[:, 0:1]
    fsp[:, :HALF] = fs[:, 0:1]
    fyp[:, HALF + HI:HALF + HI + HALF] = fy[:, -1:]
    fsp[:, HALF + HI:HALF + HI + HALF] = fs[:, -1:]

    bands = _bands_from_kernel(kernel)
    bands16 = np.ascontiguousarray(bands.T[:, :, None]).astype(np.float16)
    w1f = np.concatenate(
        [w1[:, 0:10] + w1[:, 11:21], w1[:, 10:11], w1[:, 21:22]], axis=1)
    w1t = np.stack([w1f[:, :, t9 // 3, t9 % 3].T for t9 in range(9)])
    w2t = np.stack([w2[:, :, t9 // 3, t9 % 3].T for t9 in range(9)])
    w3t = np.stack([w3[0, :, t9 // 3, t9 % 3][:, None] for t9 in range(9)])
    wtail = np.empty(NBLOB - OFF_BANDS, np.float16)
    wtail[0:OFF_W1 - OFF_BANDS] = bands16.ravel()
    wtail[OFF_W1 - OFF_BANDS:OFF_W2 - OFF_BANDS] = (
        w1t.astype(np.float16).ravel())
    wtail[OFF_W2 - OFF_BANDS:OFF_W3 - OFF_BANDS] = (
        w2t.astype(np.float16).ravel())
    wtail[OFF_W3 - OFF_BANDS:OFF_B1 - OFF_BANDS] = (
        w3t.astype(np.float16).ravel())
    wtail[OFF_B1 - OFF_BANDS:OFF_B2 - OFF_BANDS] = (
        b1.astype(np.float32).view(np.float16))
    wtail[OFF_B2 - OFF_BANDS:OFF_B3 - OFF_BANDS] = (
        b2.astype(np.float32).view(np.float16))
    wtail[OFF_B3 - OFF_BANDS:] = np.asarray(
        [b3[0] + np.float32(NS[0] / NS[1])], np.float32).view(np.float16)
    in_maps = []
    for c in range(NCORES):
        sl = slice(SW * c, SW * c + SW)
        blob = np.empty(NBLOB, np.float16)
        blob[OFF_FY:OFF_FS] = np.ascontiguousarray(
            fyp[sl].transpose(1, 0, 2)).astype(np.float16).ravel()
        blob[OFF_FS:OFF_BANDS] = np.ascontiguousarray(
            fsp[sl].transpose(1, 0, 2)).astype(np.float16).ravel()
        blob[OFF_BANDS:] = wtail
        in_maps.append(dict(blob=blob))

    res = _run(in_maps)

    # ---- assemble + fs + scatter (host)
    outs = []
    for c in range(NCORES):
        oo = res[c]["o"].astype(np.float32)
        oo = oo.reshape(SW, NT, 4, R, W2)[:, :, :, :, 1:1 + W]
        outs.append(oo.transpose(0, 2, 1, 3, 4).reshape(SW, HI, W))
    o_dev = np.concatenate(outs, axis=0)                      # [24, 1100, 52]
    out = o_dev + fs

    out_cal = np.zeros((B * P, HI, W), np.float32)
    np.add.at(out_cal, msk_idx, out)
    cnt = np.zeros((B * P,), np.float32)
    np.add.at(cnt, msk_idx, 1.0)
    out_msk = np.broadcast_to(
        (cnt > 0)[:, None, None], (B * P, HI, W)).copy()
    return (out_cal.reshape(B, P, HI, W),
            out_msk.reshape(B, P, HI, W))

